# revision 5
# baseline (speedup 1.0000x reference)
"""MoE kernel v6: 8-way F-split, all experts resident on every core.

Every core holds a distinct F/8 = 512-column slice of ALL 8 experts'
w1/w2 and processes ALL routed token columns (16384 = T*top_k) on that
slice; the 8 partial outputs are summed on host, then combined/scattered
with the router weights. Per-core PE work is exactly 16384 columns x 64
cycles regardless of the routing distribution - zero load imbalance.
(bf16 roofline: 16384 cols x 64 cyc / 2.4 GHz = 437 us.)

v6 over v5 (468.7 us): startup + tail compaction.
 - Ramp-up tiling: the first two tiles of the first expert are 256
   columns wide with small dedicated DRAM buffers (512 KB instead of
   1 MB), so the first real matmul's data lands ~9.5 us instead of
   ~14.3 us. Startup DMA issue order matches need order: x_r0, w1[e0]
   chunk 0, x_r1, w1[e0] chunks 1-3, then w2[e0] in two do-halves.
 - w2 DRAM/SBUF layout is do-major ([E, 128, KO, FLO, 128]) so mm2 of
   tile 0 can start once the first half of w2[e0] has landed.
 - Warm-up matmul count cut 26 -> 8 to match the shorter DMA wait.
 - Narrow trailing tiles (128 wide) DMA their output in 2-do-chunk
   pieces (64 KB) alternating scalar/sync as each PSUM evacuation
   lands, so the final DMA starts right after the last cast instead of
   waiting for the whole tile. Narrow tiles also read x from a packed
   256 KB buffer instead of a padded 1 MB tile.

Schedule: mm1 runs one tile ahead of mm2 (software pipeline), so mm2
never waits on its own tile's gelu. ~8 warm-up matmuls on a memset
tile occupy the PE (and warm the HAM clock gate) while the first x/w1
transfers land. The 14 MB bulk weight stream rides the gpsimd SWDGE
queue, held back by WAR anchors until tile 1 is underway so it cannot
crowd the startup-critical transfers; x rides the sync HWDGE queue and
y the scalar HWDGE queue.

DRAM layouts per core (FL = F/8 = 512, FLO = FL/128 = 4):
  x_r [n_ramp, 128, KO, RW]     bf16  ramp tiles, packed
  x   [n_big, 128, KO, CT]      bf16  x[t,p,ko,c] = xf[tok_c, ko*128+p]
  x_n [n_nar, 128, KO, TW_LAST] bf16  narrow tiles, packed
  w1  [E, 128, FLO, KO, 128]    bf16  w1[e,p,fq,ko,c] =
                                        w1_e[ko*128+p, h*FL+fq*128+c]
  w2  [E, 128, KO, FLO, 128]    bf16  w2[e,p,do,fo,c] =
                                        w2_e[h*FL+fo*128+p, do*128+c]
  b1  [128, E*FLO]              f32   b1[p, e*FLO+fo] = b1_e[h*FL+fo*128+p]
  y_r/y/y2                      bf16  partial (gelu(x@w1l+b1l) @ w2l)^T
(h = the core's F-slice index, 0..7.)
"""

import numpy as np
import ml_dtypes

N_CORES = 8
D = 1024
F = 4096
E = 8
KO = D // 128
FL = F // N_CORES    # 512 local F columns per core
FLO = FL // 128      # 4 local f-chunks
CT = 512

BF16 = ml_dtypes.bfloat16

_NC_CACHE: dict[tuple, object] = {}
LAST_RESULTS = None


RW = 256        # width of the leading ramp tiles
N_RAMP = 2      # how many leading tiles are ramp-width
TW_LAST = 128   # width of the program's trailing narrow tiles
N_NARROW = 3    # how many trailing tiles are narrow
N_WARM = 9      # warm-up matmuls on the memset tile


def _balanced_tiles(C, n_narrow=0, n_ramp=0):
    """Split C columns into tiles <= CT wide: [(off, w), ...].

    The first n_ramp tiles are exactly RW columns (fast startup), the
    final n_narrow tiles are exactly TW_LAST columns (small trailing
    DMAs); the middle is split into near-equal tiles <= CT wide.
    """
    if C <= 0:
        return []
    head_n = n_ramp if C > n_ramp * RW + 512 else 0
    tail_n = n_narrow if C - head_n * RW > n_narrow * TW_LAST + 256 else 0
    C2 = C - head_n * RW - tail_n * TW_LAST
    tiles, off = [], 0
    for _ in range(head_n):
        tiles.append((off, RW))
        off += RW
    if C2 > 0:
        n = (C2 + CT - 1) // CT
        base, rem = divmod(C2, n)
        widths = [base + 1] * rem + [base] * (n - rem)
        for w in widths:
            tiles.append((off, w))
            off += w
    for _ in range(tail_n):
        tiles.append((off, TW_LAST))
        off += TW_LAST
    return tiles


def _classify(spec):
    """-> (n_ramp, n_big, n_nar): leading RW tiles, middle, trailing."""
    n_tiles = len(spec)
    n_ramp = 0
    while n_ramp < n_tiles and spec[n_ramp][2] == RW and n_ramp < N_RAMP:
        n_ramp += 1
    n_nar = 0
    while n_nar < n_tiles - n_ramp and spec[n_tiles - 1 - n_nar][2] <= TW_LAST:
        n_nar += 1
    return n_ramp, n_tiles - n_ramp - n_nar, n_nar


def _build(spec, b1_zero):
    import concourse.mybir as mybir
    from concourse import bacc
    from concourse.tile import TileContext

    fp32 = mybir.dt.float32
    bf16 = mybir.dt.bfloat16

    n_tiles = len(spec)
    e_first = spec[0][0]
    n_ramp, n_big, n_nar = _classify(spec)

    nc = bacc.Bacc(
        "TRN2", target_bir_lowering=False, debug=False, num_devices=N_CORES
    )
    x_r = nc.dram_tensor(
        "x_r", [max(n_ramp, 1), 128, KO, RW], bf16, kind="ExternalInput"
    )
    x = nc.dram_tensor("x", [max(n_big, 1), 128, KO, CT], bf16, kind="ExternalInput")
    x_n = nc.dram_tensor(
        "x_n", [max(n_nar, 1), 128, KO, TW_LAST], bf16, kind="ExternalInput"
    )
    w1 = nc.dram_tensor("w1", [E, 128, FLO, KO, 128], bf16, kind="ExternalInput")
    w2 = nc.dram_tensor("w2", [E, 128, KO, FLO, 128], bf16, kind="ExternalInput")
    b1 = nc.dram_tensor("b1", [128, E * FLO], fp32, kind="ExternalInput")
    y_r = nc.dram_tensor(
        "y_r", [max(n_ramp, 1), 128, KO, RW], bf16, kind="ExternalOutput"
    )
    y = nc.dram_tensor("y", [max(n_big, 1), 128, KO, CT], bf16, kind="ExternalOutput")
    y2 = nc.dram_tensor(
        "y2", [max(n_nar, 1), 128, KO, TW_LAST], bf16, kind="ExternalOutput"
    )

    with TileContext(nc) as tc:
        with (
            tc.tile_pool(name="wpool", bufs=1) as wpool,
            tc.tile_pool(name="xrpool", bufs=2) as xrpool,
            tc.tile_pool(name="xpool", bufs=2) as xpool,
            tc.tile_pool(name="xnpool", bufs=2) as xnpool,
            tc.tile_pool(name="hpool", bufs=3) as hpool,
            tc.tile_pool(name="ypool", bufs=3) as ypool,
            tc.tile_pool(name="yspool", bufs=2) as yspool,
            tc.tile_pool(name="ph", bufs=4, space="PSUM") as phpool,
            tc.tile_pool(name="py", bufs=4, space="PSUM") as pypool,
        ):
            w1_sb = wpool.tile([128, E, FLO, KO, 128], bf16)
            w2_sb = wpool.tile([128, E, KO, FLO, 128], bf16)
            b1_sb = wpool.tile([128, E * FLO], fp32)
            anchor = wpool.tile([128, 16], bf16)
            others = [e for e in range(E) if e != e_first]

            # wdummy first so warm-up LDWEIGHTS/MATMULs can start ASAP.
            wdummy = wpool.tile([128, CT], bf16)
            nc.vector.memset(wdummy[:], 0.0)

            # Startup-critical transfers, ALL on the sync queue in
            # exact need-order: a single FIFO queue makes HBM serve
            # them in that order, so the critical pieces (x_r0 +
            # w1[e0] chunks) cannot be crowded out by the later, less
            # urgent ones (x_r1, w2[e0], then the loop's big x tiles
            # which follow on the same queue in program order). The
            # scalar queue carries only y stores; gpsimd carries the
            # anchored bulk weight stream.
            xr_sbs = []
            for r in range(n_ramp):
                xr_sb = xrpool.tile([128, KO, RW], bf16, tag="xr_sb")
                xr_sbs.append(xr_sb)
            x_first = None
            if n_ramp == 0:
                x_first = xpool.tile([128, KO, CT], bf16, tag="x_sb")
                nc.sync.dma_start(x_first[:, 0:4], x[0][:, 0:4])
                nc.sync.dma_start(w1_sb[:, e_first, 0], w1[e_first][:, 0])
                nc.sync.dma_start(x_first[:, 4:8], x[0][:, 4:8])
            else:
                nc.sync.dma_start(xr_sbs[0][:], x_r[0])
                nc.sync.dma_start(w1_sb[:, e_first, 0], w1[e_first][:, 0])
            for fq in range(1, FLO):
                nc.sync.dma_start(w1_sb[:, e_first, fq], w1[e_first][:, fq])
            for r in range(1, n_ramp):
                nc.sync.dma_start(xr_sbs[r][:], x_r[r])
            if b1_zero:
                nc.vector.memset(b1_sb[:], 0.0)
            else:
                nc.sync.dma_start(b1_sb[:], b1[:])
            nc.sync.dma_start(w2_sb[:, e_first, 0:4], w2[e_first][:, 0:4])
            nc.sync.dma_start(w2_sb[:, e_first, 4:8], w2[e_first][:, 4:8])

            # Dummy activation so ACT_TABLE_LOAD (Gelu tables, ~2.6 us
            # on the scalar queue) runs after the startup DMA issues
            # but well before tile 0's first real gelu.
            warm = wpool.tile([128, 1], fp32)
            nc.vector.memset(warm[:], 0.0)
            nc.scalar.activation(
                warm[:], warm[:], mybir.ActivationFunctionType.Gelu
            )

            # Warm-up matmuls on the memset tile: keep the PE busy (and
            # the HAM clock warming) while the first x/w1 chunks land.
            for _ in range(N_WARM):
                ph = phpool.tile([128, CT], fp32, tag="ph")
                nc.tensor.matmul(
                    ph[:], lhsT=wdummy[:, 0:128], rhs=wdummy[:],
                    start=True, stop=True,
                )

            def mm1_tile(ti, e, tw, x_sb):
                h_sb = hpool.tile([128, FLO, CT], bf16)
                for fo in range(FLO):
                    ph = phpool.tile([128, CT], fp32, tag="ph")
                    for ko in range(KO):
                        nc.tensor.matmul(
                            ph[:, :tw],
                            lhsT=w1_sb[:, e, fo, ko, :],
                            rhs=x_sb[:, ko, :tw],
                            start=(ko == 0),
                            stop=(ko == KO - 1),
                        )
                    nc.scalar.activation(
                        h_sb[:, fo, :tw],
                        ph[:, :tw],
                        mybir.ActivationFunctionType.Gelu,
                        bias=b1_sb[:, e * FLO + fo : e * FLO + fo + 1],
                    )
                return h_sb

            def mm2_tile(ti, e, tw, h_sb):
                ramp = ti < n_ramp
                narrow = ti >= n_tiles - n_nar
                if ramp:
                    y_sb = yspool.tile([128, KO, RW], bf16, tag="yr_sb")
                elif narrow:
                    y_sb = yspool.tile([128, KO, TW_LAST], bf16, tag="y2_sb")
                else:
                    y_sb = ypool.tile([128, KO, CT], bf16, tag="y_sb")
                for do in range(KO):
                    py = pypool.tile([128, CT], fp32)
                    for fo in range(FLO):
                        nc.tensor.matmul(
                            py[:, :tw],
                            lhsT=w2_sb[:, e, do, fo, :],
                            rhs=h_sb[:, fo, :tw],
                            start=(fo == 0),
                            stop=(fo == FLO - 1),
                        )
                    nc.vector.tensor_copy(y_sb[:, do, :tw], py[:, :tw])
                    # Narrow tiles: ship each 2-do chunk (64 KB) as soon
                    # as its casts land, alternating queues, so the last
                    # DMA starts right after the final cast.
                    if narrow and do % 2 == 1:
                        idx = ti - (n_tiles - n_nar)
                        q = do // 2
                        eng = nc.scalar if q % 2 == 0 else nc.sync
                        eng.dma_start(
                            y2[idx][:, do - 1 : do + 1], y_sb[:, do - 1 : do + 1]
                        )
                if ramp:
                    nc.scalar.dma_start(y_r[ti][:], y_sb[:])
                elif not narrow:
                    nc.scalar.dma_start(y[ti - n_ramp][:], y_sb[:])

            # Software pipeline: mm1 runs one tile ahead of mm2.
            prev = None
            for ti, (e, off, tw) in enumerate(spec):
                if ti < n_ramp:
                    x_sb = xr_sbs[ti]
                elif ti == 0:
                    x_sb = x_first
                elif ti >= n_tiles - n_nar:
                    x_sb = xnpool.tile([128, KO, TW_LAST], bf16, tag="xn_sb")
                    nc.sync.dma_start(x_sb[:], x_n[ti - (n_tiles - n_nar)])
                else:
                    x_sb = xpool.tile([128, KO, CT], bf16, tag="x_sb")
                    nc.sync.dma_start(x_sb[:], x[ti - n_ramp])
                h_sb = mm1_tile(ti, e, tw, x_sb)
                anchor_ti = 1 if n_tiles > 1 else 0
                if ti == anchor_ti:
                    # WAR anchors: tiny reads of each pending weight
                    # region, chained after this tile's first h chunk,
                    # so the scheduler cannot hoist the 14 MB bulk
                    # weight stream (gpsimd queue) into the startup
                    # window.
                    nc.vector.tensor_copy(anchor[:, 0:1], h_sb[:, 0, 0:1])
                    for k, eo in enumerate(others):
                        nc.vector.tensor_add(
                            anchor[:, 1 + k : 2 + k],
                            w1_sb[:, eo, 0, 0, 0:1],
                            anchor[:, 0:1],
                        )
                        nc.vector.tensor_add(
                            anchor[:, 8 + k : 9 + k],
                            w2_sb[:, eo, 0, 0, 0:1],
                            anchor[:, 0:1],
                        )
                    for eo in others:
                        nc.gpsimd.dma_start(w1_sb[:, eo], w1[eo])
                        nc.gpsimd.dma_start(w2_sb[:, eo], w2[eo])
                if prev is not None:
                    mm2_tile(*prev)
                prev = (ti, e, tw, h_sb)
            mm2_tile(*prev)

    nc.compile()
    return nc


def kernel(x, gate_w, w1, b1, w2, b2):
    from concourse.bass_utils import run_bass_kernel_spmd

    global LAST_RESULTS

    x = np.asarray(x, dtype=np.float32)
    gate_w = np.asarray(gate_w, dtype=np.float32)
    w1 = np.asarray(w1, dtype=np.float32)
    b1 = np.asarray(b1, dtype=np.float32)
    w2 = np.asarray(w2, dtype=np.float32)
    b2 = np.asarray(b2, dtype=np.float32)

    B, S, Din = x.shape
    assert Din == D and gate_w.shape == (D, E)
    T = B * S
    xf = x.reshape(T, D)

    # ---- Host router + dispatch ----
    logits = xf.astype(np.float64) @ gate_w.astype(np.float64)
    idx0 = np.argmax(logits, axis=1)
    rows = np.arange(T)
    v0 = logits[rows, idx0]
    l2 = logits.copy()
    l2[rows, idx0] = -np.inf
    idx1 = np.argmax(l2, axis=1)
    v1_ = l2[rows, idx1]
    e1 = np.exp(v1_ - v0)
    cw0 = 1.0 / (1.0 + e1)
    cw1 = e1 / (1.0 + e1)

    token_ids = []
    combine_w = []
    for e in range(E):
        sel0 = idx0 == e
        sel1 = idx1 == e
        ids = np.nonzero(sel0 | sel1)[0]
        w = np.where(sel0[ids], cw0[ids], cw1[ids])
        token_ids.append(ids)
        combine_w.append(w)

    spec = []
    for e in range(E):
        for off, tw in _balanced_tiles(
            len(token_ids[e]),
            n_narrow=(N_NARROW if e == E - 1 else 0),
            n_ramp=(N_RAMP if e == 0 else 0),
        ):
            spec.append((e, off, tw))
    spec = tuple(spec)
    n_tiles = len(spec)
    n_ramp, n_big, n_nar = _classify(spec)

    b1_zero = bool(np.all(b1 == 0.0))
    key = (spec, b1_zero)
    if key not in _NC_CACHE:
        _NC_CACHE[key] = _build(spec, b1_zero)
    nc = _NC_CACHE[key]

    # ---- Shared x tiles; per-core weight slices ----
    xr_tiles = np.zeros((max(n_ramp, 1), 128, KO, RW), dtype=BF16)
    xtiles = np.zeros((max(n_big, 1), 128, KO, CT), dtype=BF16)
    xn_tiles = np.zeros((max(n_nar, 1), 128, KO, TW_LAST), dtype=BF16)
    for ti, (e, off, tw) in enumerate(spec):
        ids_seg = token_ids[e][off : off + tw]
        blk = xf[ids_seg].astype(BF16).reshape(tw, KO, 128).transpose(2, 1, 0)
        if ti < n_ramp:
            xr_tiles[ti, :, :, :tw] = blk
        elif ti >= n_tiles - n_nar:
            xn_tiles[ti - (n_tiles - n_nar), :, :, :tw] = blk
        else:
            xtiles[ti - n_ramp, :, :, :tw] = blk
    xr_tiles = np.ascontiguousarray(xr_tiles)
    xtiles = np.ascontiguousarray(xtiles)
    xn_tiles = np.ascontiguousarray(xn_tiles)

    b1f = b1.astype(np.float32)
    in_maps = []
    for h in range(N_CORES):
        sl = slice(h * FL, (h + 1) * FL)
        w1c = np.stack(
            [
                w1[e][:, sl]
                .reshape(KO, 128, FLO, 128)
                .transpose(1, 2, 0, 3)
                for e in range(E)
            ]
        ).astype(BF16)  # [E, 128, FLO, KO, 128]
        w2c = np.stack(
            [
                w2[e][sl, :]
                .reshape(FLO, 128, KO, 128)
                .transpose(1, 2, 0, 3)
                for e in range(E)
            ]
        ).astype(BF16)  # [E, 128, KO, FLO, 128]
        b1c = np.stack(
            [b1f[e][sl].reshape(FLO, 128).T for e in range(E)], axis=1
        ).reshape(128, E * FLO)  # [128, E*FLO]
        in_maps.append(
            {
                "x_r": xr_tiles,
                "x": xtiles,
                "x_n": xn_tiles,
                "w1": np.ascontiguousarray(w1c),
                "w2": np.ascontiguousarray(w2c),
                "b1": np.ascontiguousarray(b1c),
            }
        )

    res = run_bass_kernel_spmd(nc, in_maps, core_ids=list(range(N_CORES)))
    LAST_RESULTS = res

    # ---- Host: sum the 8 F-slice partials, combine, scatter ----
    yr_sum = res.results[0]["y_r"].astype(np.float32)
    ysum = res.results[0]["y"].astype(np.float32)
    y2sum = res.results[0]["y2"].astype(np.float32)
    for h in range(1, N_CORES):
        yr_sum += res.results[h]["y_r"].astype(np.float32)
        ysum += res.results[h]["y"].astype(np.float32)
        y2sum += res.results[h]["y2"].astype(np.float32)

    out = np.zeros((T, D), dtype=np.float32)
    for ti, (e, off, tw) in enumerate(spec):
        ids_seg = token_ids[e][off : off + tw]
        cw_seg = combine_w[e][off : off + tw].astype(np.float32)
        if ti < n_ramp:
            yt = yr_sum[ti, :, :, :tw]
        elif ti >= n_tiles - n_nar:
            yt = y2sum[ti - (n_tiles - n_nar), :, :, :tw]
        else:
            yt = ysum[ti - n_ramp, :, :, :tw]
        yt = yt.transpose(2, 1, 0).reshape(tw, D)
        out[ids_seg] += cw_seg[:, None] * (yt + b2[e])

    return out.reshape(B, S, D)


# revision 6
# speedup vs baseline: 1.2021x; 1.2021x over previous
"""MoE kernel v8: 8-way F-split + fp8 DoubleRow for low-weight pairs.

Every core holds a distinct F/8 = 512-column slice of ALL 8 experts'
w1/w2 and processes ALL routed token columns (16384 = T*top_k) on that
slice; the 8 partial outputs are summed on host, then combined/
scattered with the router weights. Per-core PE work is independent of
the routing distribution - zero load imbalance. bf16 roofline:
16384 cols x 64 cyc / 2.4 GHz = 437 us.

v8 over v7: (token,expert) pairs whose router combine weight is below
TAU=0.35 (~17% of pairs) are computed in fp8e4m3 with
perf_mode=DoubleRow (K=256 per pass, ~2x PE throughput), cutting the
PE roofline by ~30 us. Their contribution to the output is scaled by
cw < 0.35, so the fp8 quantization error stays well inside the 2e-2
budget (simulated end-to-end rel err 1.2e-2 vs 3.8e-3 all-bf16).
Weights for the fp8 path are pre-scaled by 32 on host (into e4m3's
sweet spot) and unscaled via the gelu activation's scale=1/32 and the
host combine. Gelu emits fp8 directly (ACT converts on write).

SBUF now rotates per-expert weight slots (3 bf16 + 2 fp8 experts
resident) instead of keeping all 8 experts resident, freeing the room
for the fp8 path. Expert k+2's bf16 weights and expert k+1's fp8
weights are DMA'd (gpsimd SWDGE) when expert k begins; the slot WAR
dependencies throttle the stream automatically. Experts 1-2 (+fp8 0-1)
are issued behind a WAR anchor chained to tile 1 so the bulk cannot
crowd the startup-critical transfers.

Startup: ALL critical loads ride the sync queue in exact need-order
(x ramp tile 0, w1[e0] chunks, x ramp tile 1, w2[e0] halves, then the
loop's x tiles in program order) - a single FIFO queue makes HBM serve
them in that order. The first two tiles are 256 wide so the first
matmul's data is only 0.75 MB. ~12 warm-up matmuls on a memset tile
keep the PE busy (and the HAM clock warming) until then. y rides the
scalar queue. Narrow trailing tiles (128 wide) DMA their output in
2-do chunks alternating scalar/sync as each cast lands.
"""

import numpy as np
import ml_dtypes

N_CORES = 8
D = 1024
F = 4096
E = 8
KO = D // 128
FL = F // N_CORES    # 512 local F columns per core
FLO = FL // 128      # 4 local f-chunks
CT = 512

BF16 = ml_dtypes.bfloat16
F8 = ml_dtypes.float8_e4m3

_NC_CACHE: dict[tuple, object] = {}
LAST_RESULTS = None


RW = 256        # width of the leading ramp tiles
N_RAMP = 2      # how many leading tiles are ramp-width
TW_LAST = 128   # width of the program's trailing narrow tiles
N_NARROW = 3    # how many trailing tiles are narrow
N_WARM = 12     # warm-up matmuls on the memset tile
TAU = 0.35      # pairs with combine weight < TAU go to the fp8 path
WS = 32.0       # fp8 weight pre-scale (power of two)


def _balanced_tiles(C, n_narrow=0, n_ramp=0):
    """Split C columns into tiles <= CT wide: [(off, w), ...]."""
    if C <= 0:
        return []
    head_n = n_ramp if C > n_ramp * RW + 512 else 0
    tail_n = n_narrow if C - head_n * RW > n_narrow * TW_LAST + 256 else 0
    C2 = C - head_n * RW - tail_n * TW_LAST
    tiles, off = [], 0
    for _ in range(head_n):
        tiles.append((off, RW))
        off += RW
    if C2 > 0:
        n = (C2 + CT - 1) // CT
        base, rem = divmod(C2, n)
        widths = [base + 1] * rem + [base] * (n - rem)
        for w in widths:
            tiles.append((off, w))
            off += w
    for _ in range(tail_n):
        tiles.append((off, TW_LAST))
        off += TW_LAST
    return tiles


def _pad16(w):
    return (w + 15) & ~15


def _classify(spec):
    """-> (n_ramp, n_nar) among the bf16 tiles of spec."""
    bf = [s for s in spec if not s[3]]
    n_ramp = 0
    while n_ramp < len(bf) and bf[n_ramp][2] == RW and n_ramp < N_RAMP:
        n_ramp += 1
    n_nar = 0
    while n_nar < len(bf) - n_ramp and bf[len(bf) - 1 - n_nar][2] <= TW_LAST:
        n_nar += 1
    return n_ramp, n_nar


def _build(spec, b1_zero):
    import concourse.mybir as mybir
    from concourse import bacc
    from concourse.tile import TileContext

    fp32 = mybir.dt.float32
    bf16 = mybir.dt.bfloat16
    f8 = mybir.dt.float8e4
    DR = mybir.MatmulPerfMode.DoubleRow

    n_tiles = len(spec)
    e_first = spec[0][0]
    n_ramp, n_nar = _classify(spec)
    bf_specs = [(i, s) for i, s in enumerate(spec) if not s[3]]
    f8_specs = [(i, s) for i, s in enumerate(spec) if s[3]]
    n_bf = len(bf_specs)
    n_big = n_bf - n_ramp - n_nar
    # per-tile storage index within its class
    cls = {}
    for j, (i, s) in enumerate(bf_specs):
        if j < n_ramp:
            cls[i] = ("ramp", j)
        elif j >= n_bf - n_nar:
            cls[i] = ("nar", j - (n_bf - n_nar))
        else:
            cls[i] = ("big", j - n_ramp)
    for j, (i, s) in enumerate(f8_specs):
        cls[i] = ("f8", j)
    # experts in appearance order; expert -> has fp8 tile
    e_order = []
    for e, off, tw, is8 in spec:
        if e not in e_order:
            e_order.append(e)
    e_has8 = {e: False for e in e_order}
    for e, off, tw, is8 in spec:
        if is8:
            e_has8[e] = True
    e8_order = [e for e in e_order if e_has8[e]]

    nc = bacc.Bacc(
        "TRN2", target_bir_lowering=False, debug=False, num_devices=N_CORES
    )
    x_r = nc.dram_tensor(
        "x_r", [max(n_ramp, 1), 128, KO, RW], bf16, kind="ExternalInput"
    )
    x = nc.dram_tensor("x", [max(n_big, 1), 128, KO, CT], bf16, kind="ExternalInput")
    x_n = nc.dram_tensor(
        "x_n", [max(n_nar, 1), 128, KO, TW_LAST], bf16, kind="ExternalInput"
    )
    w1 = nc.dram_tensor("w1", [E, 128, FLO, KO, 128], bf16, kind="ExternalInput")
    w2 = nc.dram_tensor("w2", [E, 128, KO, FLO, 128], bf16, kind="ExternalInput")
    w1q = nc.dram_tensor(
        "w1q", [E, 128, FLO, KO // 2, 2, 128], f8, kind="ExternalInput"
    )
    w2q = nc.dram_tensor(
        "w2q", [E, 128, KO, FLO // 2, 2, 128], f8, kind="ExternalInput"
    )
    b1 = nc.dram_tensor("b1", [128, E * FLO], fp32, kind="ExternalInput")
    x8_d = {}
    y8_d = {}
    for j, (i, (e, off, tw, is8)) in enumerate(f8_specs):
        twp = _pad16(tw)
        x8_d[j] = nc.dram_tensor(
            f"x8_{j}", [128, KO, twp], f8, kind="ExternalInput"
        )
        y8_d[j] = nc.dram_tensor(
            f"y8_{j}", [128, KO, twp], bf16, kind="ExternalOutput"
        )
    y_r = nc.dram_tensor(
        "y_r", [max(n_ramp, 1), 128, KO, RW], bf16, kind="ExternalOutput"
    )
    y = nc.dram_tensor("y", [max(n_big, 1), 128, KO, CT], bf16, kind="ExternalOutput")
    y2 = nc.dram_tensor(
        "y2", [max(n_nar, 1), 128, KO, TW_LAST], bf16, kind="ExternalOutput"
    )

    with TileContext(nc) as tc:
        with (
            tc.tile_pool(name="cpool", bufs=1) as cpool,
            tc.tile_pool(name="wepool", bufs=3) as wepool,
            tc.tile_pool(name="w8pool", bufs=2) as w8pool,
            tc.tile_pool(name="xrpool", bufs=2) as xrpool,
            tc.tile_pool(name="xpool", bufs=2) as xpool,
            tc.tile_pool(name="xnpool", bufs=2) as xnpool,
            tc.tile_pool(name="x8pool", bufs=2) as x8pool,
            tc.tile_pool(name="hpool", bufs=3) as hpool,
            tc.tile_pool(name="h8pool", bufs=2) as h8pool,
            tc.tile_pool(name="ypool", bufs=3) as ypool,
            tc.tile_pool(name="yspool", bufs=2) as yspool,
            tc.tile_pool(name="y8pool", bufs=2) as y8pool,
            tc.tile_pool(name="ph", bufs=4, space="PSUM") as phpool,
            tc.tile_pool(name="py", bufs=4, space="PSUM") as pypool,
        ):
            b1_sb = cpool.tile([128, E * FLO], fp32)
            anchor = cpool.tile([128, 32], bf16)

            def alloc_we():
                return (
                    wepool.tile([128, FLO, KO, 128], bf16, tag="w1e"),
                    wepool.tile([128, KO, FLO, 128], bf16, tag="w2e"),
                )

            def alloc_w8():
                return (
                    w8pool.tile([128, FLO, KO // 2, 2, 128], f8, tag="w1q"),
                    w8pool.tile([128, KO, FLO // 2, 2, 128], f8, tag="w2q"),
                )

            we = {}       # expert -> (w1t, w2t)
            w8 = {}       # expert -> (w1qt, w2qt)
            we[e_order[0]] = alloc_we()
            for e in e_order[1:3]:
                we[e] = alloc_we()
            for e in e8_order[:2]:
                w8[e] = alloc_w8()

            # wdummy first so warm-up LDWEIGHTS/MATMULs can start ASAP.
            wdummy = cpool.tile([128, CT], bf16)
            nc.vector.memset(wdummy[:], 0.0)

            # Startup-critical transfers, ALL on the sync queue in
            # exact need-order (single FIFO => HBM serves in order).
            w1t0, w2t0 = we[e_first]
            xr_sbs = []
            for r in range(n_ramp):
                xr_sbs.append(xrpool.tile([128, KO, RW], bf16, tag="xr_sb"))
            x_first = None
            if n_ramp == 0:
                x_first = xpool.tile([128, KO, CT], bf16, tag="x_sb")
                nc.sync.dma_start(x_first[:, 0:4], x[0][:, 0:4])
                nc.sync.dma_start(w1t0[:, 0], w1[e_first][:, 0])
                nc.sync.dma_start(x_first[:, 4:8], x[0][:, 4:8])
            else:
                nc.sync.dma_start(xr_sbs[0][:], x_r[0])
                nc.sync.dma_start(w1t0[:, 0], w1[e_first][:, 0])
            for fq in range(1, FLO):
                nc.sync.dma_start(w1t0[:, fq], w1[e_first][:, fq])
            for r in range(1, n_ramp):
                nc.sync.dma_start(xr_sbs[r][:], x_r[r])
            if b1_zero:
                nc.vector.memset(b1_sb[:], 0.0)
            else:
                nc.sync.dma_start(b1_sb[:], b1[:])
            nc.sync.dma_start(w2t0[:, 0:4], w2[e_first][:, 0:4])
            nc.sync.dma_start(w2t0[:, 4:8], w2[e_first][:, 4:8])

            # Gelu table loads ride the scalar queue here (it carries
            # no startup DMAs), finishing before the first real gelu.
            warm = cpool.tile([128, 1], fp32)
            nc.vector.memset(warm[:], 0.0)
            nc.scalar.activation(
                warm[:], warm[:], mybir.ActivationFunctionType.Gelu
            )

            for _ in range(N_WARM):
                ph = phpool.tile([128, CT], fp32, tag="ph")
                nc.tensor.matmul(
                    ph[:], lhsT=wdummy[:, 0:128], rhs=wdummy[:],
                    start=True, stop=True,
                )

            def mm1_tile(ti, e, tw, x_sb):
                w1t = we[e][0]
                h_sb = hpool.tile([128, FLO, CT], bf16)
                for fo in range(FLO):
                    ph = phpool.tile([128, CT], fp32, tag="ph")
                    for ko in range(KO):
                        nc.tensor.matmul(
                            ph[:, :tw],
                            lhsT=w1t[:, fo, ko, :],
                            rhs=x_sb[:, ko, :tw],
                            start=(ko == 0),
                            stop=(ko == KO - 1),
                        )
                    nc.scalar.activation(
                        h_sb[:, fo, :tw],
                        ph[:, :tw],
                        mybir.ActivationFunctionType.Gelu,
                        bias=b1_sb[:, e * FLO + fo : e * FLO + fo + 1],
                    )
                return h_sb

            def mm2_tile(ti, e, tw, h_sb):
                w2t = we[e][1]
                kind, idx = cls[ti]
                if kind == "ramp":
                    y_sb = yspool.tile([128, KO, RW], bf16, tag="yr_sb")
                elif kind == "nar":
                    y_sb = yspool.tile([128, KO, TW_LAST], bf16, tag="y2_sb")
                else:
                    y_sb = ypool.tile([128, KO, CT], bf16, tag="y_sb")
                for do in range(KO):
                    py = pypool.tile([128, CT], fp32)
                    for fo in range(FLO):
                        nc.tensor.matmul(
                            py[:, :tw],
                            lhsT=w2t[:, do, fo, :],
                            rhs=h_sb[:, fo, :tw],
                            start=(fo == 0),
                            stop=(fo == FLO - 1),
                        )
                    nc.vector.tensor_copy(y_sb[:, do, :tw], py[:, :tw])
                    if kind == "nar" and do % 2 == 1:
                        q = do // 2
                        eng = nc.scalar if q % 2 == 0 else nc.sync
                        eng.dma_start(
                            y2[idx][:, do - 1 : do + 1], y_sb[:, do - 1 : do + 1]
                        )
                if kind == "ramp":
                    nc.scalar.dma_start(y_r[idx][:], y_sb[:])
                elif kind == "big":
                    nc.scalar.dma_start(y[idx][:], y_sb[:])

            def mm1_tile_f8(ti, e, tw, x8_sb, twp):
                w1qt = w8[e][0]
                h8_sb = h8pool.tile([128, FLO, twp], f8, tag="h8_sb")
                for fo in range(FLO):
                    ph = phpool.tile([128, CT], fp32, tag="ph")
                    for j in range(KO // 2):
                        nc.tensor.matmul(
                            ph[:, :twp],
                            lhsT=w1qt[:, fo, j],
                            rhs=x8_sb[:, 2 * j : 2 * j + 2, :],
                            start=(j == 0),
                            stop=(j == KO // 2 - 1),
                            perf_mode=DR,
                        )
                    nc.scalar.activation(
                        h8_sb[:, fo, :],
                        ph[:, :twp],
                        mybir.ActivationFunctionType.Gelu,
                        bias=b1_sb[:, e * FLO + fo : e * FLO + fo + 1],
                        scale=1.0 / WS,
                    )
                return h8_sb

            def mm2_tile_f8(ti, e, tw, h8_sb):
                w2qt = w8[e][1]
                kind, idx = cls[ti]
                twp = _pad16(tw)
                y_sb = y8pool.tile([128, KO, twp], bf16, tag="y8_sb")
                for do in range(KO):
                    py = pypool.tile([128, CT], fp32)
                    for q in range(FLO // 2):
                        nc.tensor.matmul(
                            py[:, :twp],
                            lhsT=w2qt[:, do, q],
                            rhs=h8_sb[:, 2 * q : 2 * q + 2, :],
                            start=(q == 0),
                            stop=(q == FLO // 2 - 1),
                            perf_mode=DR,
                        )
                    nc.vector.tensor_copy(y_sb[:, do, :], py[:, :twp])
                nc.scalar.dma_start(y8_d[idx][:], y_sb[:])

            def issue_expert_dmas(k):
                # at expert k's first tile: bf16 weights for k+2, fp8
                # weights for k+1 (slot WAR throttles automatically).
                if k + 2 < len(e_order):
                    e2 = e_order[k + 2]
                    we[e2] = alloc_we()
                    nc.gpsimd.dma_start(we[e2][0][:], w1[e2])
                    nc.gpsimd.dma_start(we[e2][1][:], w2[e2])
                nxt8 = [e for e in e8_order if e8_order.index(e) >= 2]
                kpos = [e8_order.index(e) for e in (e_order[k + 1],) if e in e8_order]
                if k + 1 < len(e_order):
                    e1 = e_order[k + 1]
                    if e_has8[e1] and e1 not in w8:
                        w8[e1] = alloc_w8()
                        nc.gpsimd.dma_start(w8[e1][0][:], w1q[e1])
                        nc.gpsimd.dma_start(w8[e1][1][:], w2q[e1])

            # Software pipeline: mm1 runs one tile ahead of mm2.
            prev = None
            cur_e_pos = 0
            for ti, (e, off, tw, is8) in enumerate(spec):
                if e != e_order[cur_e_pos]:
                    cur_e_pos += 1
                    issue_expert_dmas(cur_e_pos)
                kind, idx = cls[ti]
                if is8:
                    twp = _pad16(tw)
                    x_sb = x8pool.tile([128, KO, twp], f8, tag="x8_sb")
                    nc.sync.dma_start(x_sb[:], x8_d[idx])
                    h_sb = mm1_tile_f8(ti, e, tw, x_sb, twp)
                elif kind == "ramp":
                    x_sb = xr_sbs[idx]
                    h_sb = mm1_tile(ti, e, tw, x_sb)
                elif ti == 0:
                    x_sb = x_first
                    h_sb = mm1_tile(ti, e, tw, x_sb)
                elif kind == "nar":
                    x_sb = xnpool.tile([128, KO, TW_LAST], bf16, tag="xn_sb")
                    nc.sync.dma_start(x_sb[:], x_n[idx])
                    h_sb = mm1_tile(ti, e, tw, x_sb)
                else:
                    x_sb = xpool.tile([128, KO, CT], bf16, tag="x_sb")
                    nc.sync.dma_start(x_sb[:], x[idx])
                    h_sb = mm1_tile(ti, e, tw, x_sb)
                anchor_ti = 1 if n_tiles > 1 else 0
                if ti == anchor_ti:
                    # WAR anchors: tiny reads of each pending weight
                    # region so the scheduler cannot hoist the bulk
                    # weight stream into the startup window.
                    nc.vector.tensor_copy(anchor[:, 0:1], h_sb[:, 0, 0:1])
                    pend = []
                    for e2 in e_order[1:3]:
                        pend.append(we[e2][0][:, 0, 0, 0:1])
                        pend.append(we[e2][1][:, 0, 0, 0:1])
                    for e2 in e8_order[:2]:
                        pend.append(w8[e2][0][:, 0, 0, 0, 0:1])
                        pend.append(w8[e2][1][:, 0, 0, 0, 0:1])
                    for k, ap in enumerate(pend):
                        nc.vector.tensor_add(
                            anchor[:, 1 + k : 2 + k], ap, anchor[:, 0:1]
                        )
                    for e2 in e_order[1:3]:
                        nc.gpsimd.dma_start(we[e2][0][:], w1[e2])
                        nc.gpsimd.dma_start(we[e2][1][:], w2[e2])
                    for e2 in e8_order[:2]:
                        nc.gpsimd.dma_start(w8[e2][0][:], w1q[e2])
                        nc.gpsimd.dma_start(w8[e2][1][:], w2q[e2])
                if prev is not None:
                    pti, pe, ptw, ph_sb, pis8 = prev
                    if pis8:
                        mm2_tile_f8(pti, pe, ptw, ph_sb)
                    else:
                        mm2_tile(pti, pe, ptw, ph_sb)
                prev = (ti, e, tw, h_sb, is8)
            pti, pe, ptw, ph_sb, pis8 = prev
            if pis8:
                mm2_tile_f8(pti, pe, ptw, ph_sb)
            else:
                mm2_tile(pti, pe, ptw, ph_sb)

    nc.compile()
    return nc


def kernel(x, gate_w, w1, b1, w2, b2):
    from concourse.bass_utils import run_bass_kernel_spmd

    global LAST_RESULTS

    x = np.asarray(x, dtype=np.float32)
    gate_w = np.asarray(gate_w, dtype=np.float32)
    w1 = np.asarray(w1, dtype=np.float32)
    b1 = np.asarray(b1, dtype=np.float32)
    w2 = np.asarray(w2, dtype=np.float32)
    b2 = np.asarray(b2, dtype=np.float32)

    B, S, Din = x.shape
    assert Din == D and gate_w.shape == (D, E)
    T = B * S
    xf = x.reshape(T, D)

    # ---- Host router + dispatch ----
    logits = xf.astype(np.float64) @ gate_w.astype(np.float64)
    idx0 = np.argmax(logits, axis=1)
    rows = np.arange(T)
    v0 = logits[rows, idx0]
    l2 = logits.copy()
    l2[rows, idx0] = -np.inf
    idx1 = np.argmax(l2, axis=1)
    v1_ = l2[rows, idx1]
    e1 = np.exp(v1_ - v0)
    cw0 = 1.0 / (1.0 + e1)
    cw1 = e1 / (1.0 + e1)

    token_ids = []     # bf16 pairs per expert
    combine_w = []
    token_ids8 = []    # fp8 pairs per expert
    combine_w8 = []
    for e in range(E):
        sel0 = idx0 == e
        sel1 = idx1 == e
        ids = np.nonzero(sel0 | sel1)[0]
        w = np.where(sel0[ids], cw0[ids], cw1[ids])
        m8 = w < TAU
        # tiny fp8 groups aren't worth a tile
        if m8.sum() < 64:
            m8[:] = False
        token_ids.append(ids[~m8])
        combine_w.append(w[~m8])
        token_ids8.append(ids[m8])
        combine_w8.append(w[m8])

    spec = []
    for e in range(E):
        bf_tiles = _balanced_tiles(
            len(token_ids[e]),
            n_narrow=(N_NARROW if e == E - 1 else 0),
            n_ramp=(N_RAMP if e == 0 else 0),
        )
        f8_tiles = _balanced_tiles(len(token_ids8[e]))
        if e == E - 1:
            n_nar_e = 0
            while n_nar_e < len(bf_tiles) and bf_tiles[len(bf_tiles) - 1 - n_nar_e][1] <= TW_LAST:
                n_nar_e += 1
            big_part = bf_tiles[: len(bf_tiles) - n_nar_e]
            nar_part = bf_tiles[len(bf_tiles) - n_nar_e :]
            for off, tw in big_part:
                spec.append((e, off, tw, False))
            for off, tw in f8_tiles:
                spec.append((e, off, tw, True))
            for off, tw in nar_part:
                spec.append((e, off, tw, False))
        else:
            for off, tw in bf_tiles:
                spec.append((e, off, tw, False))
            for off, tw in f8_tiles:
                spec.append((e, off, tw, True))
    spec = tuple(spec)
    n_tiles = len(spec)
    n_ramp, n_nar = _classify(spec)
    bf_specs = [(i, s) for i, s in enumerate(spec) if not s[3]]
    f8_specs = [(i, s) for i, s in enumerate(spec) if s[3]]
    n_big = len(bf_specs) - n_ramp - n_nar

    b1_zero = bool(np.all(b1 == 0.0))
    key = (spec, b1_zero)
    if key not in _NC_CACHE:
        _NC_CACHE[key] = _build(spec, b1_zero)
    nc = _NC_CACHE[key]

    # ---- Shared x tiles; per-core weight slices ----
    xr_tiles = np.zeros((max(n_ramp, 1), 128, KO, RW), dtype=BF16)
    xtiles = np.zeros((max(n_big, 1), 128, KO, CT), dtype=BF16)
    xn_tiles = np.zeros((max(n_nar, 1), 128, KO, TW_LAST), dtype=BF16)
    x8_tiles = {}
    jbf = 0
    j8 = 0
    for ti, (e, off, tw, is8) in enumerate(spec):
        if is8:
            ids_seg = token_ids8[e][off : off + tw]
            twp = _pad16(tw)
            blk = np.zeros((128, KO, twp), dtype=F8)
            xq = np.clip(xf[ids_seg], -240, 240).astype(F8)
            blk[:, :, :tw] = xq.reshape(tw, KO, 128).transpose(2, 1, 0)
            x8_tiles[f"x8_{j8}"] = np.ascontiguousarray(blk)
            j8 += 1
            continue
        ids_seg = token_ids[e][off : off + tw]
        blk = xf[ids_seg].astype(BF16).reshape(tw, KO, 128).transpose(2, 1, 0)
        if jbf < n_ramp:
            xr_tiles[jbf, :, :, :tw] = blk
        elif jbf >= len(bf_specs) - n_nar:
            xn_tiles[jbf - (len(bf_specs) - n_nar), :, :, :tw] = blk
        else:
            xtiles[jbf - n_ramp, :, :, :tw] = blk
        jbf += 1
    xr_tiles = np.ascontiguousarray(xr_tiles)
    xtiles = np.ascontiguousarray(xtiles)
    xn_tiles = np.ascontiguousarray(xn_tiles)

    b1f = b1.astype(np.float32)
    in_maps = []
    for h in range(N_CORES):
        sl = slice(h * FL, (h + 1) * FL)
        w1c = np.stack(
            [
                w1[e][:, sl]
                .reshape(KO, 128, FLO, 128)
                .transpose(1, 2, 0, 3)
                for e in range(E)
            ]
        ).astype(BF16)  # [E, 128, FLO, KO, 128]
        w2c = np.stack(
            [
                w2[e][sl, :]
                .reshape(FLO, 128, KO, 128)
                .transpose(1, 2, 0, 3)
                for e in range(E)
            ]
        ).astype(BF16)  # [E, 128, KO, FLO, 128]
        # fp8 copies, pre-scaled by WS, DoubleRow-pair layouts
        w1qc = np.stack(
            [
                np.clip(w1[e][:, sl] * WS, -240, 240)
                .astype(F8)
                .reshape(KO // 2, 2, 128, FLO, 128)
                .transpose(2, 3, 0, 1, 4)
                for e in range(E)
            ]
        )  # [E, 128, FLO, KO//2, 2, 128]
        w2qc = np.stack(
            [
                np.clip(w2[e][sl, :] * WS, -240, 240)
                .astype(F8)
                .reshape(FLO // 2, 2, 128, KO, 128)
                .transpose(2, 3, 0, 1, 4)
                for e in range(E)
            ]
        )  # [E, 128, KO, FLO//2, 2, 128]
        b1c = np.stack(
            [b1f[e][sl].reshape(FLO, 128).T for e in range(E)], axis=1
        ).reshape(128, E * FLO)  # [128, E*FLO]
        m = {
            "x_r": xr_tiles,
            "x": xtiles,
            "x_n": xn_tiles,
            "w1": np.ascontiguousarray(w1c),
            "w2": np.ascontiguousarray(w2c),
            "w1q": np.ascontiguousarray(w1qc),
            "w2q": np.ascontiguousarray(w2qc),
            "b1": np.ascontiguousarray(b1c),
        }
        m.update(x8_tiles)
        in_maps.append(m)

    res = run_bass_kernel_spmd(nc, in_maps, core_ids=list(range(N_CORES)))
    LAST_RESULTS = res

    # ---- Host: sum the 8 F-slice partials, combine, scatter ----
    def summed(name):
        s = res.results[0][name].astype(np.float32)
        for h in range(1, N_CORES):
            s = s + res.results[h][name].astype(np.float32)
        return s

    yr_sum = summed("y_r")
    ysum = summed("y")
    y2sum = summed("y2")
    y8sum = {j: summed(f"y8_{j}") for j in range(len(f8_specs))}

    out = np.zeros((T, D), dtype=np.float32)
    jbf = 0
    j8 = 0
    for ti, (e, off, tw, is8) in enumerate(spec):
        if is8:
            ids_seg = token_ids8[e][off : off + tw]
            cw_seg = combine_w8[e][off : off + tw].astype(np.float32)
            yt = y8sum[j8][:, :, :tw].transpose(2, 1, 0).reshape(tw, D)
            out[ids_seg] += cw_seg[:, None] * (yt * np.float32(1.0 / WS) + b2[e])
            j8 += 1
            continue
        ids_seg = token_ids[e][off : off + tw]
        cw_seg = combine_w[e][off : off + tw].astype(np.float32)
        if jbf < n_ramp:
            yt = yr_sum[jbf, :, :, :tw]
        elif jbf >= len(bf_specs) - n_nar:
            yt = y2sum[jbf - (len(bf_specs) - n_nar), :, :, :tw]
        else:
            yt = ysum[jbf - n_ramp, :, :, :tw]
        yt = yt.transpose(2, 1, 0).reshape(tw, D)
        out[ids_seg] += cw_seg[:, None] * (yt + b2[e])
        jbf += 1

    return out.reshape(B, S, D)


# revision 10
# speedup vs baseline: 1.2658x; 1.0530x over previous
"""MoE kernel v8: 8-way F-split + fp8 DoubleRow for low-weight pairs.

Every core holds a distinct F/8 = 512-column slice of ALL 8 experts'
w1/w2 and processes ALL routed token columns (16384 = T*top_k) on that
slice; the 8 partial outputs are summed on host, then combined/
scattered with the router weights. Per-core PE work is independent of
the routing distribution - zero load imbalance. bf16 roofline:
16384 cols x 64 cyc / 2.4 GHz = 437 us.

v8 over v7: (token,expert) pairs whose router combine weight is below
TAU=0.35 (~17% of pairs) are computed in fp8e4m3 with
perf_mode=DoubleRow (K=256 per pass, ~2x PE throughput), cutting the
PE roofline by ~30 us. Their contribution to the output is scaled by
cw < 0.35, so the fp8 quantization error stays well inside the 2e-2
budget (simulated end-to-end rel err 1.2e-2 vs 3.8e-3 all-bf16).
Weights for the fp8 path are pre-scaled by 32 on host (into e4m3's
sweet spot) and unscaled via the gelu activation's scale=1/32 and the
host combine. Gelu emits fp8 directly (ACT converts on write).

SBUF now rotates per-expert weight slots (3 bf16 + 2 fp8 experts
resident) instead of keeping all 8 experts resident, freeing the room
for the fp8 path. Expert k+2's bf16 weights and expert k+1's fp8
weights are DMA'd (gpsimd SWDGE) when expert k begins; the slot WAR
dependencies throttle the stream automatically. Experts 1-2 (+fp8 0-1)
are issued behind a WAR anchor chained to tile 1 so the bulk cannot
crowd the startup-critical transfers.

Startup: ALL critical loads ride the sync queue in exact need-order
(x ramp tile 0, w1[e0] chunks, x ramp tile 1, w2[e0] halves, then the
loop's x tiles in program order) - a single FIFO queue makes HBM serve
them in that order. The first two tiles are 256 wide so the first
matmul's data is only 0.75 MB. ~12 warm-up matmuls on a memset tile
keep the PE busy (and the HAM clock warming) until then. y rides the
scalar queue. Narrow trailing tiles (128 wide) DMA their output in
2-do chunks alternating scalar/sync as each cast lands.
"""

import numpy as np
import ml_dtypes

N_CORES = 8
D = 1024
F = 4096
E = 8
KO = D // 128
FL = F // N_CORES    # 512 local F columns per core
FLO = FL // 128      # 4 local f-chunks
CT = 512

BF16 = ml_dtypes.bfloat16
F8 = ml_dtypes.float8_e4m3

_NC_CACHE: dict[tuple, object] = {}
LAST_RESULTS = None


RW = 256        # width of the leading ramp tiles
N_RAMP = 2      # how many leading tiles are ramp-width
TW_LAST = 128   # width of the program's trailing narrow tiles
N_NARROW = 3    # how many trailing tiles are narrow
N_WARM = 12     # warm-up matmuls on the memset tile
TAU = 0.35      # pairs with combine weight < TAU go to the fp8 path
WS = 32.0       # fp8 weight pre-scale (power of two)


def _balanced_tiles(C, n_narrow=0, n_ramp=0):
    """Split C columns into tiles <= CT wide: [(off, w), ...]."""
    if C <= 0:
        return []
    head_n = n_ramp if C > n_ramp * RW + 512 else 0
    tail_n = n_narrow if C - head_n * RW > n_narrow * TW_LAST + 256 else 0
    C2 = C - head_n * RW - tail_n * TW_LAST
    tiles, off = [], 0
    for _ in range(head_n):
        tiles.append((off, RW))
        off += RW
    if C2 > 0:
        n = (C2 + CT - 1) // CT
        base, rem = divmod(C2, n)
        widths = [base + 1] * rem + [base] * (n - rem)
        for w in widths:
            tiles.append((off, w))
            off += w
    for _ in range(tail_n):
        tiles.append((off, TW_LAST))
        off += TW_LAST
    return tiles


def _pad16(w):
    return (w + 15) & ~15


def _classify(spec):
    """-> (n_ramp, n_nar) among the bf16 tiles of spec."""
    bf = [s for s in spec if not s[3]]
    n_ramp = 0
    while n_ramp < len(bf) and bf[n_ramp][2] == RW and n_ramp < N_RAMP:
        n_ramp += 1
    n_nar = 0
    while n_nar < len(bf) - n_ramp and bf[len(bf) - 1 - n_nar][2] <= TW_LAST:
        n_nar += 1
    return n_ramp, n_nar


def _build(spec, b1_zero):
    import concourse.mybir as mybir
    from concourse import bacc
    from concourse.tile import TileContext

    fp32 = mybir.dt.float32
    bf16 = mybir.dt.bfloat16
    f8 = mybir.dt.float8e4
    DR = mybir.MatmulPerfMode.DoubleRow

    n_tiles = len(spec)
    e_first = spec[0][0]
    n_ramp, n_nar = _classify(spec)
    bf_specs = [(i, s) for i, s in enumerate(spec) if not s[3]]
    f8_specs = [(i, s) for i, s in enumerate(spec) if s[3]]
    n_bf = len(bf_specs)
    n_big = n_bf - n_ramp - n_nar
    # per-tile storage index within its class
    cls = {}
    for j, (i, s) in enumerate(bf_specs):
        if j < n_ramp:
            cls[i] = ("ramp", j)
        elif j >= n_bf - n_nar:
            cls[i] = ("nar", j - (n_bf - n_nar))
        else:
            cls[i] = ("big", j - n_ramp)
    for j, (i, s) in enumerate(f8_specs):
        cls[i] = ("f8", j)
    # experts in appearance order; expert -> has fp8 tile
    e_order = []
    for e, off, tw, is8 in spec:
        if e not in e_order:
            e_order.append(e)
    e_has8 = {e: False for e in e_order}
    for e, off, tw, is8 in spec:
        if is8:
            e_has8[e] = True
    e8_order = [e for e in e_order if e_has8[e]]

    nc = bacc.Bacc(
        "TRN2", target_bir_lowering=False, debug=False, num_devices=N_CORES
    )
    x_r = nc.dram_tensor(
        "x_r", [max(n_ramp, 1), 128, KO, RW], bf16, kind="ExternalInput"
    )
    x = nc.dram_tensor("x", [max(n_big, 1), 128, KO, CT], bf16, kind="ExternalInput")
    x_n = nc.dram_tensor(
        "x_n", [max(n_nar, 1), 128, KO, TW_LAST], bf16, kind="ExternalInput"
    )
    w1 = nc.dram_tensor("w1", [E, 128, FLO, KO, 128], bf16, kind="ExternalInput")
    w2 = nc.dram_tensor("w2", [E, 128, KO, FLO, 128], bf16, kind="ExternalInput")
    w1q = nc.dram_tensor(
        "w1q", [E, 128, FLO, KO // 2, 2, 128], f8, kind="ExternalInput"
    )
    w2q = nc.dram_tensor(
        "w2q", [E, 128, KO, FLO // 2, 2, 128], f8, kind="ExternalInput"
    )
    b1 = nc.dram_tensor("b1", [128, E * FLO], fp32, kind="ExternalInput")
    x8_d = {}
    y8_d = {}
    for j, (i, (e, off, tw, is8)) in enumerate(f8_specs):
        twp = _pad16(tw)
        x8_d[j] = nc.dram_tensor(
            f"x8_{j}", [128, KO, twp], f8, kind="ExternalInput"
        )
        y8_d[j] = nc.dram_tensor(
            f"y8_{j}", [128, KO, twp], bf16, kind="ExternalOutput"
        )
    y_r = nc.dram_tensor(
        "y_r", [max(n_ramp, 1), 128, KO, RW], bf16, kind="ExternalOutput"
    )
    y = nc.dram_tensor("y", [max(n_big, 1), 128, KO, CT], bf16, kind="ExternalOutput")
    y2 = nc.dram_tensor(
        "y2", [max(n_nar, 1), 128, KO, TW_LAST], bf16, kind="ExternalOutput"
    )

    with TileContext(nc) as tc:
        with (
            tc.tile_pool(name="cpool", bufs=1) as cpool,
            tc.tile_pool(name="wepool", bufs=3) as wepool,
            tc.tile_pool(name="w8pool", bufs=2) as w8pool,
            tc.tile_pool(name="xrpool", bufs=2) as xrpool,
            tc.tile_pool(name="xpool", bufs=2) as xpool,
            tc.tile_pool(name="xnpool", bufs=2) as xnpool,
            tc.tile_pool(name="x8pool", bufs=2) as x8pool,
            tc.tile_pool(name="hpool", bufs=3) as hpool,
            tc.tile_pool(name="h8pool", bufs=2) as h8pool,
            tc.tile_pool(name="ypool", bufs=3) as ypool,
            tc.tile_pool(name="yspool", bufs=2) as yspool,
            tc.tile_pool(name="y8pool", bufs=2) as y8pool,
            tc.tile_pool(name="ph", bufs=4, space="PSUM") as phpool,
            tc.tile_pool(name="py", bufs=4, space="PSUM") as pypool,
        ):
            b1_sb = cpool.tile([128, E * FLO], fp32)
            anchor = cpool.tile([128, 32], bf16)

            def alloc_we():
                w1t = wepool.tile([128, FLO, KO, 128], bf16, tag="w1e")
                w2t = wepool.tile([128, KO, FLO, 128], bf16, tag="w2e")
                return w1t, w2t

            def alloc_w8():
                w1qt = w8pool.tile([128, FLO, KO // 2, 2, 128], f8, tag="w1q")
                w2qt = w8pool.tile([128, KO, FLO // 2, 2, 128], f8, tag="w2q")
                return w1qt, w2qt

            we = {}       # expert -> (w1t, w2t)
            w8 = {}       # expert -> (w1qt, w2qt)
            we[e_order[0]] = alloc_we()
            for e in e_order[1:3]:
                we[e] = alloc_we()
            for e in e8_order[:2]:
                w8[e] = alloc_w8()

            # wdummy first so warm-up LDWEIGHTS/MATMULs can start ASAP.
            wdummy = cpool.tile([128, CT], bf16)
            nc.vector.memset(wdummy[:], 0.0)

            # Startup-critical transfers, ALL on the sync queue in
            # exact need-order (single FIFO => HBM serves in order).
            w1t0, w2t0 = we[e_first]
            xr_sbs = []
            for r in range(n_ramp):
                xr_sb = xrpool.tile([128, KO, RW], bf16, tag="xr_sb")
                xr_sbs.append(xr_sb)
            x_first = None
            if n_ramp == 0:
                x_first = xpool.tile([128, KO, CT], bf16, tag="x_sb")
                nc.sync.dma_start(x_first[:, 0:4], x[0][:, 0:4])
                nc.sync.dma_start(w1t0[:, 0], w1[e_first][:, 0])
                nc.sync.dma_start(x_first[:, 4:8], x[0][:, 4:8])
            else:
                nc.sync.dma_start(xr_sbs[0][:], x_r[0])
                nc.sync.dma_start(w1t0[:, 0], w1[e_first][:, 0])
            for fq in range(1, FLO):
                nc.sync.dma_start(w1t0[:, fq], w1[e_first][:, fq])
            for r in range(1, n_ramp):
                nc.sync.dma_start(xr_sbs[r][:], x_r[r])
            if b1_zero:
                nc.vector.memset(b1_sb[:], 0.0)
            else:
                nc.sync.dma_start(b1_sb[:], b1[:])
            nc.sync.dma_start(w2t0[:, 0:4], w2[e_first][:, 0:4])
            nc.sync.dma_start(w2t0[:, 4:8], w2[e_first][:, 4:8])

            # Gelu table loads ride the scalar queue here (it carries
            # no startup DMAs), finishing before the first real gelu.
            warm = cpool.tile([128, 1], fp32)
            nc.vector.memset(warm[:], 0.0)
            nc.scalar.activation(
                warm[:], warm[:], mybir.ActivationFunctionType.Gelu
            )

            for _ in range(N_WARM):
                ph = phpool.tile([128, CT], fp32, tag="ph")
                nc.tensor.matmul(
                    ph[:], lhsT=wdummy[:, 0:128], rhs=wdummy[:],
                    start=True, stop=True,
                )

            def mm1_tile(ti, e, tw, x_sb):
                w1t = we[e][0]
                h_sb = hpool.tile([128, FLO, CT], bf16)
                for fo in range(FLO):
                    ph = phpool.tile([128, CT], fp32, tag="ph")
                    for ko in range(KO):
                        nc.tensor.matmul(
                            ph[:, :tw],
                            lhsT=w1t[:, fo, ko, :],
                            rhs=x_sb[:, ko, :tw],
                            start=(ko == 0),
                            stop=(ko == KO - 1),
                        )
                    nc.scalar.activation(
                        h_sb[:, fo, :tw],
                        ph[:, :tw],
                        mybir.ActivationFunctionType.Gelu,
                        bias=b1_sb[:, e * FLO + fo : e * FLO + fo + 1],
                    )
                return h_sb

            def mm2_tile(ti, e, tw, h_sb):
                w2t = we[e][1]
                kind, idx = cls[ti]
                if kind == "ramp":
                    y_sb = yspool.tile([128, KO, RW], bf16, tag="yr_sb")
                elif kind == "nar":
                    y_sb = yspool.tile([128, KO, TW_LAST], bf16, tag="y2_sb")
                else:
                    y_sb = ypool.tile([128, KO, CT], bf16, tag="y_sb")
                for do in range(KO):
                    py = pypool.tile([128, CT], fp32)
                    for fo in range(FLO):
                        nc.tensor.matmul(
                            py[:, :tw],
                            lhsT=w2t[:, do, fo, :],
                            rhs=h_sb[:, fo, :tw],
                            start=(fo == 0),
                            stop=(fo == FLO - 1),
                        )
                    nc.vector.tensor_copy(y_sb[:, do, :tw], py[:, :tw])
                    if kind == "nar" and do % 2 == 1:
                        q = do // 2
                        eng = nc.scalar if q % 2 == 0 else nc.sync
                        eng.dma_start(
                            y2[idx][:, do - 1 : do + 1], y_sb[:, do - 1 : do + 1]
                        )
                if kind == "ramp":
                    nc.scalar.dma_start(y_r[idx][:], y_sb[:])
                elif kind == "big":
                    nc.scalar.dma_start(y[idx][:], y_sb[:])

            def mm1_tile_f8(ti, e, tw, x8_sb, twp):
                w1qt = w8[e][0]
                h8_sb = h8pool.tile([128, FLO, twp], f8, tag="h8_sb")
                for fo in range(FLO):
                    ph = phpool.tile([128, CT], fp32, tag="ph")
                    for j in range(KO // 2):
                        nc.tensor.matmul(
                            ph[:, :twp],
                            lhsT=w1qt[:, fo, j],
                            rhs=x8_sb[:, 2 * j : 2 * j + 2, :],
                            start=(j == 0),
                            stop=(j == KO // 2 - 1),
                            perf_mode=DR,
                        )
                    nc.scalar.activation(
                        h8_sb[:, fo, :],
                        ph[:, :twp],
                        mybir.ActivationFunctionType.Gelu,
                        bias=b1_sb[:, e * FLO + fo : e * FLO + fo + 1],
                        scale=1.0 / WS,
                    )
                return h8_sb

            def mm2_tile_f8(ti, e, tw, h8_sb):
                w2qt = w8[e][1]
                kind, idx = cls[ti]
                twp = _pad16(tw)
                y_sb = y8pool.tile([128, KO, twp], bf16, tag="y8_sb")
                for do in range(KO):
                    py = pypool.tile([128, CT], fp32)
                    for q in range(FLO // 2):
                        nc.tensor.matmul(
                            py[:, :twp],
                            lhsT=w2qt[:, do, q],
                            rhs=h8_sb[:, 2 * q : 2 * q + 2, :],
                            start=(q == 0),
                            stop=(q == FLO // 2 - 1),
                            perf_mode=DR,
                        )
                    nc.vector.tensor_copy(y_sb[:, do, :], py[:, :twp])
                nc.scalar.dma_start(y8_d[idx][:], y_sb[:])

            def issue_expert_dmas(k):
                # at expert k's first tile: bf16 weights for k+2, fp8
                # weights for k+1 (slot WAR throttles automatically).
                if k + 2 < len(e_order):
                    e2 = e_order[k + 2]
                    we[e2] = alloc_we()
                    nc.gpsimd.dma_start(we[e2][0][:], w1[e2])
                    nc.gpsimd.dma_start(we[e2][1][:], w2[e2])
                if k + 1 < len(e_order):
                    e1 = e_order[k + 1]
                    if e_has8[e1] and e1 not in w8:
                        w8[e1] = alloc_w8()
                        nc.gpsimd.dma_start(w8[e1][0][:], w1q[e1])
                        nc.gpsimd.dma_start(w8[e1][1][:], w2q[e1])

            # Software pipeline: mm1 runs one tile ahead of mm2.
            prev = None
            cur_e_pos = 0
            for ti, (e, off, tw, is8) in enumerate(spec):
                if e != e_order[cur_e_pos]:
                    cur_e_pos += 1
                    issue_expert_dmas(cur_e_pos)
                kind, idx = cls[ti]
                if is8:
                    twp = _pad16(tw)
                    x_sb = x8pool.tile([128, KO, twp], f8, tag="x8_sb")
                    nc.sync.dma_start(x_sb[:], x8_d[idx][:])
                    h_sb = mm1_tile_f8(ti, e, tw, x_sb, twp)
                elif kind == "ramp":
                    x_sb = xr_sbs[idx]
                    h_sb = mm1_tile(ti, e, tw, x_sb)
                elif ti == 0:
                    x_sb = x_first
                    h_sb = mm1_tile(ti, e, tw, x_sb)
                elif kind == "nar":
                    x_sb = xnpool.tile([128, KO, TW_LAST], bf16, tag="xn_sb")
                    nc.sync.dma_start(x_sb[:], x_n[idx])
                    h_sb = mm1_tile(ti, e, tw, x_sb)
                else:
                    x_sb = xpool.tile([128, KO, CT], bf16, tag="x_sb")
                    nc.sync.dma_start(x_sb[:], x[idx])
                    h_sb = mm1_tile(ti, e, tw, x_sb)
                anchor_ti = 1 if n_tiles > 1 else 0
                if ti == anchor_ti:
                    # WAR anchors: tiny reads of each pending weight
                    # region so the scheduler cannot hoist the bulk
                    # weight stream into the startup window.
                    nc.vector.tensor_copy(anchor[:, 0:1], h_sb[:, 0, 0:1])
                    pend = []
                    for e2 in e_order[1:3]:
                        pend.append(we[e2][0][:, 0, 0, 0:1])
                        pend.append(we[e2][1][:, 0, 0, 0:1])
                    for e2 in e8_order[:2]:
                        pend.append(w8[e2][0][:, 0, 0, 0, 0:1])
                        pend.append(w8[e2][1][:, 0, 0, 0, 0:1])
                    for k, ap in enumerate(pend):
                        nc.vector.tensor_add(
                            anchor[:, 1 + k : 2 + k], ap, anchor[:, 0:1]
                        )
                    for e2 in e_order[1:3]:
                        nc.gpsimd.dma_start(we[e2][0][:], w1[e2])
                        nc.gpsimd.dma_start(we[e2][1][:], w2[e2])
                    for e2 in e8_order[:2]:
                        nc.gpsimd.dma_start(w8[e2][0][:], w1q[e2])
                        nc.gpsimd.dma_start(w8[e2][1][:], w2q[e2])
                if prev is not None:
                    pti, pe, ptw, ph_sb, pis8 = prev
                    if pis8:
                        mm2_tile_f8(pti, pe, ptw, ph_sb)
                    else:
                        mm2_tile(pti, pe, ptw, ph_sb)
                prev = (ti, e, tw, h_sb, is8)
            pti, pe, ptw, ph_sb, pis8 = prev
            if pis8:
                mm2_tile_f8(pti, pe, ptw, ph_sb)
            else:
                mm2_tile(pti, pe, ptw, ph_sb)

    nc.compile()
    return nc


def kernel(x, gate_w, w1, b1, w2, b2):
    from concourse.bass_utils import run_bass_kernel_spmd

    global LAST_RESULTS

    x = np.asarray(x, dtype=np.float32)
    gate_w = np.asarray(gate_w, dtype=np.float32)
    w1 = np.asarray(w1, dtype=np.float32)
    b1 = np.asarray(b1, dtype=np.float32)
    w2 = np.asarray(w2, dtype=np.float32)
    b2 = np.asarray(b2, dtype=np.float32)

    B, S, Din = x.shape
    assert Din == D and gate_w.shape == (D, E)
    T = B * S
    xf = x.reshape(T, D)

    # ---- Host router + dispatch ----
    logits = xf.astype(np.float64) @ gate_w.astype(np.float64)
    idx0 = np.argmax(logits, axis=1)
    rows = np.arange(T)
    v0 = logits[rows, idx0]
    l2 = logits.copy()
    l2[rows, idx0] = -np.inf
    idx1 = np.argmax(l2, axis=1)
    v1_ = l2[rows, idx1]
    e1 = np.exp(v1_ - v0)
    cw0 = 1.0 / (1.0 + e1)
    cw1 = e1 / (1.0 + e1)

    token_ids = []     # bf16 pairs per expert
    combine_w = []
    token_ids8 = []    # fp8 pairs per expert
    combine_w8 = []
    for e in range(E):
        sel0 = idx0 == e
        sel1 = idx1 == e
        ids = np.nonzero(sel0 | sel1)[0]
        w = np.where(sel0[ids], cw0[ids], cw1[ids])
        m8 = w < TAU
        # tiny fp8 groups aren't worth a tile
        if m8.sum() < 64:
            m8[:] = False
        token_ids.append(ids[~m8])
        combine_w.append(w[~m8])
        token_ids8.append(ids[m8])
        combine_w8.append(w[m8])

    spec = []
    for e in range(E):
        bf_tiles = _balanced_tiles(
            len(token_ids[e]),
            n_narrow=(N_NARROW if e == E - 1 else 0),
            n_ramp=(N_RAMP if e == 0 else 0),
        )
        f8_tiles = _balanced_tiles(len(token_ids8[e]))
        if e == E - 1:
            n_nar_e = 0
            while n_nar_e < len(bf_tiles) and bf_tiles[len(bf_tiles) - 1 - n_nar_e][1] <= TW_LAST:
                n_nar_e += 1
            big_part = bf_tiles[: len(bf_tiles) - n_nar_e]
            nar_part = bf_tiles[len(bf_tiles) - n_nar_e :]
            for off, tw in big_part:
                spec.append((e, off, tw, False))
            for off, tw in f8_tiles:
                spec.append((e, off, tw, True))
            for off, tw in nar_part:
                spec.append((e, off, tw, False))
        else:
            for off, tw in bf_tiles:
                spec.append((e, off, tw, False))
            for off, tw in f8_tiles:
                spec.append((e, off, tw, True))
    spec = tuple(spec)
    n_tiles = len(spec)
    n_ramp, n_nar = _classify(spec)
    bf_specs = [(i, s) for i, s in enumerate(spec) if not s[3]]
    f8_specs = [(i, s) for i, s in enumerate(spec) if s[3]]
    n_big = len(bf_specs) - n_ramp - n_nar

    b1_zero = bool(np.all(b1 == 0.0))
    key = (spec, b1_zero)
    if key not in _NC_CACHE:
        _NC_CACHE[key] = _build(spec, b1_zero)
    nc = _NC_CACHE[key]

    # ---- Shared x tiles; per-core weight slices ----
    xr_tiles = np.zeros((max(n_ramp, 1), 128, KO, RW), dtype=BF16)
    xtiles = np.zeros((max(n_big, 1), 128, KO, CT), dtype=BF16)
    xn_tiles = np.zeros((max(n_nar, 1), 128, KO, TW_LAST), dtype=BF16)
    x8_tiles = {}
    jbf = 0
    j8 = 0
    for ti, (e, off, tw, is8) in enumerate(spec):
        if is8:
            ids_seg = token_ids8[e][off : off + tw]
            twp = _pad16(tw)
            blk = np.zeros((128, KO, twp), dtype=F8)
            xq = np.clip(xf[ids_seg], -240, 240).astype(F8)
            blk[:, :, :tw] = xq.reshape(tw, KO, 128).transpose(2, 1, 0)
            x8_tiles[f"x8_{j8}"] = np.ascontiguousarray(blk)
            j8 += 1
            continue
        ids_seg = token_ids[e][off : off + tw]
        blk = xf[ids_seg].astype(BF16).reshape(tw, KO, 128).transpose(2, 1, 0)
        if jbf < n_ramp:
            xr_tiles[jbf, :, :, :tw] = blk
        elif jbf >= len(bf_specs) - n_nar:
            xn_tiles[jbf - (len(bf_specs) - n_nar), :, :, :tw] = blk
        else:
            xtiles[jbf - n_ramp, :, :, :tw] = blk
        jbf += 1
    xr_tiles = np.ascontiguousarray(xr_tiles)
    xtiles = np.ascontiguousarray(xtiles)
    xn_tiles = np.ascontiguousarray(xn_tiles)

    b1f = b1.astype(np.float32)
    in_maps = []
    for h in range(N_CORES):
        sl = slice(h * FL, (h + 1) * FL)
        w1c = np.stack(
            [
                w1[e][:, sl]
                .reshape(KO, 128, FLO, 128)
                .transpose(1, 2, 0, 3)
                for e in range(E)
            ]
        ).astype(BF16)  # [E, 128, FLO, KO, 128]
        w2c = np.stack(
            [
                w2[e][sl, :]
                .reshape(FLO, 128, KO, 128)
                .transpose(1, 2, 0, 3)
                for e in range(E)
            ]
        ).astype(BF16)  # [E, 128, KO, FLO, 128]
        # fp8 copies, pre-scaled by WS, DoubleRow-pair layouts
        w1qc = np.stack(
            [
                np.clip(w1[e][:, sl] * WS, -240, 240)
                .astype(F8)
                .reshape(KO // 2, 2, 128, FLO, 128)
                .transpose(2, 3, 0, 1, 4)
                for e in range(E)
            ]
        )  # [E, 128, FLO, KO//2, 2, 128]
        w2qc = np.stack(
            [
                np.clip(w2[e][sl, :] * WS, -240, 240)
                .astype(F8)
                .reshape(FLO // 2, 2, 128, KO, 128)
                .transpose(2, 3, 0, 1, 4)
                for e in range(E)
            ]
        )  # [E, 128, KO, FLO//2, 2, 128]
        b1c = np.stack(
            [b1f[e][sl].reshape(FLO, 128).T for e in range(E)], axis=1
        ).reshape(128, E * FLO)  # [128, E*FLO]
        m = {
            "x_r": xr_tiles,
            "x": xtiles,
            "x_n": xn_tiles,
            "w1": np.ascontiguousarray(w1c),
            "w2": np.ascontiguousarray(w2c),
            "w1q": np.ascontiguousarray(w1qc),
            "w2q": np.ascontiguousarray(w2qc),
            "b1": np.ascontiguousarray(b1c),
        }
        m.update(x8_tiles)
        in_maps.append(m)

    res = run_bass_kernel_spmd(nc, in_maps, core_ids=list(range(N_CORES)))
    LAST_RESULTS = res

    # ---- Host: sum the 8 F-slice partials, combine, scatter ----
    def summed(name):
        s = res.results[0][name].astype(np.float32)
        for h in range(1, N_CORES):
            s = s + res.results[h][name].astype(np.float32)
        return s

    yr_sum = summed("y_r")
    ysum = summed("y")
    y2sum = summed("y2")
    y8sum = {j: summed(f"y8_{j}") for j in range(len(f8_specs))}

    out = np.zeros((T, D), dtype=np.float32)
    jbf = 0
    j8 = 0
    for ti, (e, off, tw, is8) in enumerate(spec):
        if is8:
            ids_seg = token_ids8[e][off : off + tw]
            cw_seg = combine_w8[e][off : off + tw].astype(np.float32)
            yt = y8sum[j8][:, :, :tw].transpose(2, 1, 0).reshape(tw, D)
            out[ids_seg] += cw_seg[:, None] * (yt * np.float32(1.0 / WS) + b2[e])
            j8 += 1
            continue
        ids_seg = token_ids[e][off : off + tw]
        cw_seg = combine_w[e][off : off + tw].astype(np.float32)
        if jbf < n_ramp:
            yt = yr_sum[jbf, :, :, :tw]
        elif jbf >= len(bf_specs) - n_nar:
            yt = y2sum[jbf - (len(bf_specs) - n_nar), :, :, :tw]
        else:
            yt = ysum[jbf - n_ramp, :, :, :tw]
        yt = yt.transpose(2, 1, 0).reshape(tw, D)
        out[ids_seg] += cw_seg[:, None] * (yt + b2[e])
        jbf += 1

    return out.reshape(B, S, D)


# revision 14
# speedup vs baseline: 1.2666x; 1.0006x over previous
"""MoE kernel v8: 8-way F-split + fp8 DoubleRow for low-weight pairs.

Every core holds a distinct F/8 = 512-column slice of ALL 8 experts'
w1/w2 and processes ALL routed token columns (16384 = T*top_k) on that
slice; the 8 partial outputs are summed on host, then combined/
scattered with the router weights. Per-core PE work is independent of
the routing distribution - zero load imbalance. bf16 roofline:
16384 cols x 64 cyc / 2.4 GHz = 437 us.

v8 over v7: (token,expert) pairs whose router combine weight is below
TAU=0.35 (~17% of pairs) are computed in fp8e4m3 with
perf_mode=DoubleRow (K=256 per pass, ~2x PE throughput), cutting the
PE roofline by ~30 us. Their contribution to the output is scaled by
cw < 0.35, so the fp8 quantization error stays well inside the 2e-2
budget (simulated end-to-end rel err 1.2e-2 vs 3.8e-3 all-bf16).
Weights for the fp8 path are pre-scaled by 32 on host (into e4m3's
sweet spot) and unscaled via the gelu activation's scale=1/32 and the
host combine. Gelu emits fp8 directly (ACT converts on write).

SBUF now rotates per-expert weight slots (3 bf16 + 2 fp8 experts
resident) instead of keeping all 8 experts resident, freeing the room
for the fp8 path. Expert k+2's bf16 weights and expert k+1's fp8
weights are DMA'd (gpsimd SWDGE) when expert k begins; the slot WAR
dependencies throttle the stream automatically. Experts 1-2 (+fp8 0-1)
are issued behind a WAR anchor chained to tile 1 so the bulk cannot
crowd the startup-critical transfers.

Startup: ALL critical loads ride the sync queue in exact need-order
(x ramp tile 0, w1[e0] chunks, x ramp tile 1, w2[e0] halves, then the
loop's x tiles in program order) - a single FIFO queue makes HBM serve
them in that order. The first two tiles are 256 wide so the first
matmul's data is only 0.75 MB. ~12 warm-up matmuls on a memset tile
keep the PE busy (and the HAM clock warming) until then. y rides the
scalar queue. Narrow trailing tiles (128 wide) DMA their output in
2-do chunks alternating scalar/sync as each cast lands.
"""

import numpy as np
import ml_dtypes

N_CORES = 8
D = 1024
F = 4096
E = 8
KO = D // 128
FL = F // N_CORES    # 512 local F columns per core
FLO = FL // 128      # 4 local f-chunks
CT = 512

BF16 = ml_dtypes.bfloat16
F8 = ml_dtypes.float8_e4m3

_NC_CACHE: dict[tuple, object] = {}
LAST_RESULTS = None


RW = 256        # width of the leading ramp tiles
N_RAMP = 2      # how many leading tiles are ramp-width
TW_LAST = 128   # width of the program's trailing narrow tiles
N_NARROW = 3    # how many trailing tiles are narrow
N_WARM = 12     # warm-up matmuls on the memset tile
TAU = 0.35      # pairs with combine weight < TAU go to the fp8 path
WS = 32.0       # fp8 weight pre-scale (power of two)


def _balanced_tiles(C, n_narrow=0, n_ramp=0):
    """Split C columns into tiles <= CT wide: [(off, w), ...]."""
    if C <= 0:
        return []
    head_n = n_ramp if C > n_ramp * RW + 512 else 0
    tail_n = n_narrow if C - head_n * RW > n_narrow * TW_LAST + 256 else 0
    C2 = C - head_n * RW - tail_n * TW_LAST
    tiles, off = [], 0
    for _ in range(head_n):
        tiles.append((off, RW))
        off += RW
    if C2 > 0:
        n = (C2 + CT - 1) // CT
        base, rem = divmod(C2, n)
        widths = [base + 1] * rem + [base] * (n - rem)
        for w in widths:
            tiles.append((off, w))
            off += w
    for _ in range(tail_n):
        tiles.append((off, TW_LAST))
        off += TW_LAST
    return tiles


def _pad16(w):
    return (w + 15) & ~15


def _classify(spec):
    """-> (n_ramp, n_nar) among the bf16 tiles of spec."""
    bf = [s for s in spec if not s[3]]
    n_ramp = 0
    while n_ramp < len(bf) and bf[n_ramp][2] == RW and n_ramp < N_RAMP:
        n_ramp += 1
    n_nar = 0
    while n_nar < len(bf) - n_ramp and bf[len(bf) - 1 - n_nar][2] <= TW_LAST:
        n_nar += 1
    return n_ramp, n_nar


def _build(spec, b1_zero):
    import concourse.mybir as mybir
    from concourse import bacc
    from concourse.tile import TileContext

    fp32 = mybir.dt.float32
    bf16 = mybir.dt.bfloat16
    f8 = mybir.dt.float8e4
    DR = mybir.MatmulPerfMode.DoubleRow

    n_tiles = len(spec)
    e_first = spec[0][0]
    n_ramp, n_nar = _classify(spec)
    bf_specs = [(i, s) for i, s in enumerate(spec) if not s[3]]
    f8_specs = [(i, s) for i, s in enumerate(spec) if s[3]]
    n_bf = len(bf_specs)
    n_big = n_bf - n_ramp - n_nar
    # per-tile storage index within its class
    cls = {}
    for j, (i, s) in enumerate(bf_specs):
        if j < n_ramp:
            cls[i] = ("ramp", j)
        elif j >= n_bf - n_nar:
            cls[i] = ("nar", j - (n_bf - n_nar))
        else:
            cls[i] = ("big", j - n_ramp)
    for j, (i, s) in enumerate(f8_specs):
        cls[i] = ("f8", j)
    # experts in appearance order; expert -> has fp8 tile
    e_order = []
    for e, off, tw, is8 in spec:
        if e not in e_order:
            e_order.append(e)
    e_has8 = {e: False for e in e_order}
    for e, off, tw, is8 in spec:
        if is8:
            e_has8[e] = True
    e8_order = [e for e in e_order if e_has8[e]]

    nc = bacc.Bacc(
        "TRN2", target_bir_lowering=False, debug=False, num_devices=N_CORES
    )
    x_r = nc.dram_tensor(
        "x_r", [max(n_ramp, 1), 128, KO, RW], bf16, kind="ExternalInput"
    )
    x = nc.dram_tensor("x", [max(n_big, 1), 128, KO, CT], bf16, kind="ExternalInput")
    x_n = nc.dram_tensor(
        "x_n", [max(n_nar, 1), 128, KO, TW_LAST], bf16, kind="ExternalInput"
    )
    w1 = nc.dram_tensor("w1", [E, 128, FLO, KO, 128], bf16, kind="ExternalInput")
    w2 = nc.dram_tensor("w2", [E, 128, KO, FLO, 128], bf16, kind="ExternalInput")
    w1q = nc.dram_tensor(
        "w1q", [E, 128, FLO, KO // 2, 2, 128], f8, kind="ExternalInput"
    )
    w2q = nc.dram_tensor(
        "w2q", [E, 128, KO, FLO // 2, 2, 128], f8, kind="ExternalInput"
    )
    b1 = nc.dram_tensor("b1", [128, E * FLO], fp32, kind="ExternalInput")
    x8_d = {}
    y8_d = {}
    for j, (i, (e, off, tw, is8)) in enumerate(f8_specs):
        twp = _pad16(tw)
        x8_d[j] = nc.dram_tensor(
            f"x8_{j}", [128, KO, twp], f8, kind="ExternalInput"
        )
        y8_d[j] = nc.dram_tensor(
            f"y8_{j}", [128, KO, twp], bf16, kind="ExternalOutput"
        )
    y_r = nc.dram_tensor(
        "y_r", [max(n_ramp, 1), 128, KO, RW], bf16, kind="ExternalOutput"
    )
    y = nc.dram_tensor("y", [max(n_big, 1), 128, KO, CT], bf16, kind="ExternalOutput")
    y2 = nc.dram_tensor(
        "y2", [max(n_nar, 1), 128, KO, TW_LAST], bf16, kind="ExternalOutput"
    )

    with TileContext(nc) as tc:
        with (
            tc.tile_pool(name="cpool", bufs=1) as cpool,
            tc.tile_pool(name="wepool", bufs=3) as wepool,
            tc.tile_pool(name="w8pool", bufs=2) as w8pool,
            tc.tile_pool(name="xrpool", bufs=2) as xrpool,
            tc.tile_pool(name="xpool", bufs=2) as xpool,
            tc.tile_pool(name="xnpool", bufs=2) as xnpool,
            tc.tile_pool(name="x8pool", bufs=2) as x8pool,
            tc.tile_pool(name="hpool", bufs=3) as hpool,
            tc.tile_pool(name="h8pool", bufs=2) as h8pool,
            tc.tile_pool(name="ypool", bufs=3) as ypool,
            tc.tile_pool(name="yspool", bufs=2) as yspool,
            tc.tile_pool(name="y8pool", bufs=2) as y8pool,
            tc.tile_pool(name="ph", bufs=4, space="PSUM") as phpool,
            tc.tile_pool(name="py", bufs=4, space="PSUM") as pypool,
        ):
            b1_sb = cpool.tile([128, E * FLO], fp32)
            anchor = cpool.tile([128, 32], bf16)

            def alloc_we():
                w1t = wepool.tile([128, FLO, KO, 128], bf16, tag="w1e")
                w2t = wepool.tile([128, KO, FLO, 128], bf16, tag="w2e")
                return w1t, w2t

            def alloc_w8():
                w1qt = w8pool.tile([128, FLO, KO // 2, 2, 128], f8, tag="w1q")
                w2qt = w8pool.tile([128, KO, FLO // 2, 2, 128], f8, tag="w2q")
                return w1qt, w2qt

            we = {}       # expert -> (w1t, w2t)
            w8 = {}       # expert -> (w1qt, w2qt)
            we[e_order[0]] = alloc_we()
            for e in e_order[1:3]:
                we[e] = alloc_we()
            for e in e8_order[:2]:
                w8[e] = alloc_w8()

            # wdummy first so warm-up LDWEIGHTS/MATMULs can start ASAP.
            wdummy = cpool.tile([128, CT], bf16)
            nc.vector.memset(wdummy[:], 0.0)

            # Startup-critical transfers, ALL on the sync queue in
            # exact need-order (single FIFO => HBM serves in order).
            w1t0, w2t0 = we[e_first]
            xr_sbs = []
            for r in range(n_ramp):
                xr_sb = xrpool.tile([128, KO, RW], bf16, tag="xr_sb")
                xr_sbs.append(xr_sb)
            x_first = None
            if n_ramp == 0:
                x_first = xpool.tile([128, KO, CT], bf16, tag="x_sb")
                nc.sync.dma_start(x_first[:, 0:4], x[0][:, 0:4])
                nc.sync.dma_start(w1t0[:, 0], w1[e_first][:, 0])
                nc.sync.dma_start(x_first[:, 4:8], x[0][:, 4:8])
            else:
                nc.sync.dma_start(xr_sbs[0][:], x_r[0])
                nc.sync.dma_start(w1t0[:, 0], w1[e_first][:, 0])
            for fq in range(1, FLO):
                nc.sync.dma_start(w1t0[:, fq], w1[e_first][:, fq])
            for r in range(1, n_ramp):
                nc.sync.dma_start(xr_sbs[r][:], x_r[r])
            if b1_zero:
                nc.vector.memset(b1_sb[:], 0.0)
            else:
                nc.sync.dma_start(b1_sb[:], b1[:])
            nc.sync.dma_start(w2t0[:, 0:4], w2[e_first][:, 0:4])
            nc.sync.dma_start(w2t0[:, 4:8], w2[e_first][:, 4:8])

            # Gelu table loads ride the scalar queue here (it carries
            # no startup DMAs), finishing before the first real gelu.
            warm = cpool.tile([128, 1], fp32)
            nc.vector.memset(warm[:], 0.0)
            nc.scalar.activation(
                warm[:], warm[:], mybir.ActivationFunctionType.Gelu
            )

            for _ in range(N_WARM):
                ph = phpool.tile([128, CT], fp32, tag="ph")
                nc.tensor.matmul(
                    ph[:], lhsT=wdummy[:, 0:128], rhs=wdummy[:],
                    start=True, stop=True,
                )

            def mm1_tile(ti, e, tw, x_sb):
                w1t = we[e][0]
                h_sb = hpool.tile([128, FLO, CT], bf16)
                for fo in range(FLO):
                    ph = phpool.tile([128, CT], fp32, tag="ph")
                    for ko in range(KO):
                        nc.tensor.matmul(
                            ph[:, :tw],
                            lhsT=w1t[:, fo, ko, :],
                            rhs=x_sb[:, ko, :tw],
                            start=(ko == 0),
                            stop=(ko == KO - 1),
                        )
                    nc.scalar.activation(
                        h_sb[:, fo, :tw],
                        ph[:, :tw],
                        mybir.ActivationFunctionType.Gelu,
                        bias=b1_sb[:, e * FLO + fo : e * FLO + fo + 1],
                    )
                return h_sb

            def mm2_tile(ti, e, tw, h_sb):
                w2t = we[e][1]
                kind, idx = cls[ti]
                if kind == "ramp":
                    y_sb = yspool.tile([128, KO, RW], bf16, tag="yr_sb")
                elif kind == "nar":
                    y_sb = yspool.tile([128, KO, TW_LAST], bf16, tag="y2_sb")
                else:
                    y_sb = ypool.tile([128, KO, CT], bf16, tag="y_sb")
                for do in range(KO):
                    py = pypool.tile([128, CT], fp32)
                    for fo in range(FLO):
                        nc.tensor.matmul(
                            py[:, :tw],
                            lhsT=w2t[:, do, fo, :],
                            rhs=h_sb[:, fo, :tw],
                            start=(fo == 0),
                            stop=(fo == FLO - 1),
                        )
                    nc.vector.tensor_copy(y_sb[:, do, :tw], py[:, :tw])
                    if kind == "nar" and do % 2 == 1:
                        q = do // 2
                        eng = nc.scalar if q % 2 == 0 else nc.sync
                        eng.dma_start(
                            y2[idx][:, do - 1 : do + 1], y_sb[:, do - 1 : do + 1]
                        )
                if kind == "ramp":
                    nc.scalar.dma_start(y_r[idx][:], y_sb[:])
                elif kind == "big":
                    nc.scalar.dma_start(y[idx][:], y_sb[:])

            def mm1_tile_f8(ti, e, tw, x8_sb, twp):
                w1qt = w8[e][0]
                h8_sb = h8pool.tile([128, FLO, twp], f8, tag="h8_sb")
                for fo in range(FLO):
                    ph = phpool.tile([128, CT], fp32, tag="ph")
                    for j in range(KO // 2):
                        nc.tensor.matmul(
                            ph[:, :twp],
                            lhsT=w1qt[:, fo, j],
                            rhs=x8_sb[:, 2 * j : 2 * j + 2, :],
                            start=(j == 0),
                            stop=(j == KO // 2 - 1),
                            perf_mode=DR,
                        )
                    nc.scalar.activation(
                        h8_sb[:, fo, :],
                        ph[:, :twp],
                        mybir.ActivationFunctionType.Gelu,
                        bias=b1_sb[:, e * FLO + fo : e * FLO + fo + 1],
                        scale=1.0 / WS,
                    )
                return h8_sb

            def mm2_tile_f8(ti, e, tw, h8_sb):
                w2qt = w8[e][1]
                kind, idx = cls[ti]
                twp = _pad16(tw)
                y_sb = y8pool.tile([128, KO, twp], bf16, tag="y8_sb")
                for do in range(KO):
                    py = pypool.tile([128, CT], fp32)
                    for q in range(FLO // 2):
                        nc.tensor.matmul(
                            py[:, :twp],
                            lhsT=w2qt[:, do, q],
                            rhs=h8_sb[:, 2 * q : 2 * q + 2, :],
                            start=(q == 0),
                            stop=(q == FLO // 2 - 1),
                            perf_mode=DR,
                        )
                    nc.vector.tensor_copy(y_sb[:, do, :], py[:, :twp])
                nc.scalar.dma_start(y8_d[idx][:], y_sb[:])

            w8_issued = set()

            def issue_expert_dmas(k):
                # At expert k's first tile, in need-order: fp8 weights
                # for k+1 (needed at k+1's tail) BEFORE bf16 weights
                # for k+2 (slot WAR throttles automatically).
                if k + 1 < len(e_order):
                    e1 = e_order[k + 1]
                    if e_has8[e1] and e1 not in w8_issued:
                        if e1 not in w8:
                            w8[e1] = alloc_w8()
                        w8_issued.add(e1)
                        nc.gpsimd.dma_start(w8[e1][0][:], w1q[e1])
                        nc.gpsimd.dma_start(w8[e1][1][:], w2q[e1])
                if k + 2 < len(e_order):
                    e2 = e_order[k + 2]
                    we[e2] = alloc_we()
                    nc.gpsimd.dma_start(we[e2][0][:], w1[e2])
                    nc.gpsimd.dma_start(we[e2][1][:], w2[e2])

            # Software pipeline: mm1 runs one tile ahead of mm2.
            prev = None
            cur_e_pos = 0
            for ti, (e, off, tw, is8) in enumerate(spec):
                if e != e_order[cur_e_pos]:
                    cur_e_pos += 1
                    issue_expert_dmas(cur_e_pos)
                kind, idx = cls[ti]
                if is8:
                    twp = _pad16(tw)
                    x_sb = x8pool.tile([128, KO, twp], f8, tag="x8_sb")
                    nc.sync.dma_start(x_sb[:], x8_d[idx][:])
                    h_sb = mm1_tile_f8(ti, e, tw, x_sb, twp)
                elif kind == "ramp":
                    x_sb = xr_sbs[idx]
                    h_sb = mm1_tile(ti, e, tw, x_sb)
                elif ti == 0:
                    x_sb = x_first
                    h_sb = mm1_tile(ti, e, tw, x_sb)
                elif kind == "nar":
                    x_sb = xnpool.tile([128, KO, TW_LAST], bf16, tag="xn_sb")
                    nc.sync.dma_start(x_sb[:], x_n[idx])
                    h_sb = mm1_tile(ti, e, tw, x_sb)
                else:
                    x_sb = xpool.tile([128, KO, CT], bf16, tag="x_sb")
                    nc.sync.dma_start(x_sb[:], x[idx])
                    h_sb = mm1_tile(ti, e, tw, x_sb)
                anchor_ti = 1 if n_tiles > 1 else 0
                if ti == anchor_ti:
                    # WAR anchors: tiny reads of each pending weight
                    # region so the scheduler cannot hoist the bulk
                    # weight stream into the startup window.
                    nc.vector.tensor_copy(anchor[:, 0:1], h_sb[:, 0, 0:1])
                    pend = []
                    for e2 in e_order[1:3]:
                        pend.append(we[e2][0][:, 0, 0, 0:1])
                        pend.append(we[e2][1][:, 0, 0, 0:1])
                    for e2 in e8_order[:2]:
                        pend.append(w8[e2][0][:, 0, 0, 0, 0:1])
                        pend.append(w8[e2][1][:, 0, 0, 0, 0:1])
                    for k, ap in enumerate(pend):
                        nc.vector.tensor_add(
                            anchor[:, 1 + k : 2 + k], ap, anchor[:, 0:1]
                        )
                    # Bulk stream in need-order: e0's fp8 weights are
                    # needed first (its fp8 tile ends expert 0's span,
                    # ~tile 5), then e1 bf16, e1 fp8, e2 bf16.
                    if e8_order[:1] and e8_order[0] == e_order[0]:
                        e2 = e8_order[0]
                        w8_issued.add(e2)
                        nc.gpsimd.dma_start(w8[e2][0][:], w1q[e2])
                        nc.gpsimd.dma_start(w8[e2][1][:], w2q[e2])
                    if len(e_order) > 1:
                        e2 = e_order[1]
                        nc.gpsimd.dma_start(we[e2][0][:], w1[e2])
                        nc.gpsimd.dma_start(we[e2][1][:], w2[e2])
                        if e2 in w8:
                            w8_issued.add(e2)
                            nc.gpsimd.dma_start(w8[e2][0][:], w1q[e2])
                            nc.gpsimd.dma_start(w8[e2][1][:], w2q[e2])
                    if len(e_order) > 2:
                        e2 = e_order[2]
                        nc.gpsimd.dma_start(we[e2][0][:], w1[e2])
                        nc.gpsimd.dma_start(we[e2][1][:], w2[e2])
                if prev is not None:
                    pti, pe, ptw, ph_sb, pis8 = prev
                    if pis8:
                        mm2_tile_f8(pti, pe, ptw, ph_sb)
                    else:
                        mm2_tile(pti, pe, ptw, ph_sb)
                prev = (ti, e, tw, h_sb, is8)
            pti, pe, ptw, ph_sb, pis8 = prev
            if pis8:
                mm2_tile_f8(pti, pe, ptw, ph_sb)
            else:
                mm2_tile(pti, pe, ptw, ph_sb)

    nc.compile()
    return nc


def kernel(x, gate_w, w1, b1, w2, b2):
    from concourse.bass_utils import run_bass_kernel_spmd

    global LAST_RESULTS

    x = np.asarray(x, dtype=np.float32)
    gate_w = np.asarray(gate_w, dtype=np.float32)
    w1 = np.asarray(w1, dtype=np.float32)
    b1 = np.asarray(b1, dtype=np.float32)
    w2 = np.asarray(w2, dtype=np.float32)
    b2 = np.asarray(b2, dtype=np.float32)

    B, S, Din = x.shape
    assert Din == D and gate_w.shape == (D, E)
    T = B * S
    xf = x.reshape(T, D)

    # ---- Host router + dispatch ----
    logits = xf.astype(np.float64) @ gate_w.astype(np.float64)
    idx0 = np.argmax(logits, axis=1)
    rows = np.arange(T)
    v0 = logits[rows, idx0]
    l2 = logits.copy()
    l2[rows, idx0] = -np.inf
    idx1 = np.argmax(l2, axis=1)
    v1_ = l2[rows, idx1]
    e1 = np.exp(v1_ - v0)
    cw0 = 1.0 / (1.0 + e1)
    cw1 = e1 / (1.0 + e1)

    token_ids = []     # bf16 pairs per expert
    combine_w = []
    token_ids8 = []    # fp8 pairs per expert
    combine_w8 = []
    for e in range(E):
        sel0 = idx0 == e
        sel1 = idx1 == e
        ids = np.nonzero(sel0 | sel1)[0]
        w = np.where(sel0[ids], cw0[ids], cw1[ids])
        m8 = w < TAU
        # tiny fp8 groups aren't worth a tile
        if m8.sum() < 64:
            m8[:] = False
        token_ids.append(ids[~m8])
        combine_w.append(w[~m8])
        token_ids8.append(ids[m8])
        combine_w8.append(w[m8])

    spec = []
    for e in range(E):
        bf_tiles = _balanced_tiles(
            len(token_ids[e]),
            n_narrow=(N_NARROW if e == E - 1 else 0),
            n_ramp=(N_RAMP if e == 0 else 0),
        )
        f8_tiles = _balanced_tiles(len(token_ids8[e]))
        if e == E - 1:
            n_nar_e = 0
            while n_nar_e < len(bf_tiles) and bf_tiles[len(bf_tiles) - 1 - n_nar_e][1] <= TW_LAST:
                n_nar_e += 1
            big_part = bf_tiles[: len(bf_tiles) - n_nar_e]
            nar_part = bf_tiles[len(bf_tiles) - n_nar_e :]
            for off, tw in big_part:
                spec.append((e, off, tw, False))
            for off, tw in f8_tiles:
                spec.append((e, off, tw, True))
            for off, tw in nar_part:
                spec.append((e, off, tw, False))
        else:
            for off, tw in bf_tiles:
                spec.append((e, off, tw, False))
            for off, tw in f8_tiles:
                spec.append((e, off, tw, True))
    spec = tuple(spec)
    n_tiles = len(spec)
    n_ramp, n_nar = _classify(spec)
    bf_specs = [(i, s) for i, s in enumerate(spec) if not s[3]]
    f8_specs = [(i, s) for i, s in enumerate(spec) if s[3]]
    n_big = len(bf_specs) - n_ramp - n_nar

    b1_zero = bool(np.all(b1 == 0.0))
    key = (spec, b1_zero)
    if key not in _NC_CACHE:
        _NC_CACHE[key] = _build(spec, b1_zero)
    nc = _NC_CACHE[key]

    # ---- Shared x tiles; per-core weight slices ----
    xr_tiles = np.zeros((max(n_ramp, 1), 128, KO, RW), dtype=BF16)
    xtiles = np.zeros((max(n_big, 1), 128, KO, CT), dtype=BF16)
    xn_tiles = np.zeros((max(n_nar, 1), 128, KO, TW_LAST), dtype=BF16)
    x8_tiles = {}
    jbf = 0
    j8 = 0
    for ti, (e, off, tw, is8) in enumerate(spec):
        if is8:
            ids_seg = token_ids8[e][off : off + tw]
            twp = _pad16(tw)
            blk = np.zeros((128, KO, twp), dtype=F8)
            xq = np.clip(xf[ids_seg], -240, 240).astype(F8)
            blk[:, :, :tw] = xq.reshape(tw, KO, 128).transpose(2, 1, 0)
            x8_tiles[f"x8_{j8}"] = np.ascontiguousarray(blk)
            j8 += 1
            continue
        ids_seg = token_ids[e][off : off + tw]
        blk = xf[ids_seg].astype(BF16).reshape(tw, KO, 128).transpose(2, 1, 0)
        if jbf < n_ramp:
            xr_tiles[jbf, :, :, :tw] = blk
        elif jbf >= len(bf_specs) - n_nar:
            xn_tiles[jbf - (len(bf_specs) - n_nar), :, :, :tw] = blk
        else:
            xtiles[jbf - n_ramp, :, :, :tw] = blk
        jbf += 1
    xr_tiles = np.ascontiguousarray(xr_tiles)
    xtiles = np.ascontiguousarray(xtiles)
    xn_tiles = np.ascontiguousarray(xn_tiles)

    b1f = b1.astype(np.float32)
    in_maps = []
    for h in range(N_CORES):
        sl = slice(h * FL, (h + 1) * FL)
        w1c = np.stack(
            [
                w1[e][:, sl]
                .reshape(KO, 128, FLO, 128)
                .transpose(1, 2, 0, 3)
                for e in range(E)
            ]
        ).astype(BF16)  # [E, 128, FLO, KO, 128]
        w2c = np.stack(
            [
                w2[e][sl, :]
                .reshape(FLO, 128, KO, 128)
                .transpose(1, 2, 0, 3)
                for e in range(E)
            ]
        ).astype(BF16)  # [E, 128, KO, FLO, 128]
        # fp8 copies, pre-scaled by WS, DoubleRow-pair layouts
        w1qc = np.stack(
            [
                np.clip(w1[e][:, sl] * WS, -240, 240)
                .astype(F8)
                .reshape(KO // 2, 2, 128, FLO, 128)
                .transpose(2, 3, 0, 1, 4)
                for e in range(E)
            ]
        )  # [E, 128, FLO, KO//2, 2, 128]
        w2qc = np.stack(
            [
                np.clip(w2[e][sl, :] * WS, -240, 240)
                .astype(F8)
                .reshape(FLO // 2, 2, 128, KO, 128)
                .transpose(2, 3, 0, 1, 4)
                for e in range(E)
            ]
        )  # [E, 128, KO, FLO//2, 2, 128]
        b1c = np.stack(
            [b1f[e][sl].reshape(FLO, 128).T for e in range(E)], axis=1
        ).reshape(128, E * FLO)  # [128, E*FLO]
        m = {
            "x_r": xr_tiles,
            "x": xtiles,
            "x_n": xn_tiles,
            "w1": np.ascontiguousarray(w1c),
            "w2": np.ascontiguousarray(w2c),
            "w1q": np.ascontiguousarray(w1qc),
            "w2q": np.ascontiguousarray(w2qc),
            "b1": np.ascontiguousarray(b1c),
        }
        m.update(x8_tiles)
        in_maps.append(m)

    res = run_bass_kernel_spmd(nc, in_maps, core_ids=list(range(N_CORES)))
    LAST_RESULTS = res

    # ---- Host: sum the 8 F-slice partials, combine, scatter ----
    def summed(name):
        s = res.results[0][name].astype(np.float32)
        for h in range(1, N_CORES):
            s = s + res.results[h][name].astype(np.float32)
        return s

    yr_sum = summed("y_r")
    ysum = summed("y")
    y2sum = summed("y2")
    y8sum = {j: summed(f"y8_{j}") for j in range(len(f8_specs))}

    out = np.zeros((T, D), dtype=np.float32)
    jbf = 0
    j8 = 0
    for ti, (e, off, tw, is8) in enumerate(spec):
        if is8:
            ids_seg = token_ids8[e][off : off + tw]
            cw_seg = combine_w8[e][off : off + tw].astype(np.float32)
            yt = y8sum[j8][:, :, :tw].transpose(2, 1, 0).reshape(tw, D)
            out[ids_seg] += cw_seg[:, None] * (yt * np.float32(1.0 / WS) + b2[e])
            j8 += 1
            continue
        ids_seg = token_ids[e][off : off + tw]
        cw_seg = combine_w[e][off : off + tw].astype(np.float32)
        if jbf < n_ramp:
            yt = yr_sum[jbf, :, :, :tw]
        elif jbf >= len(bf_specs) - n_nar:
            yt = y2sum[jbf - (len(bf_specs) - n_nar), :, :, :tw]
        else:
            yt = ysum[jbf - n_ramp, :, :, :tw]
        yt = yt.transpose(2, 1, 0).reshape(tw, D)
        out[ids_seg] += cw_seg[:, None] * (yt + b2[e])
        jbf += 1

    return out.reshape(B, S, D)


# revision 17
# speedup vs baseline: 1.2969x; 1.0240x over previous
"""MoE kernel v8: 8-way F-split + fp8 DoubleRow for low-weight pairs.

Every core holds a distinct F/8 = 512-column slice of ALL 8 experts'
w1/w2 and processes ALL routed token columns (16384 = T*top_k) on that
slice; the 8 partial outputs are summed on host, then combined/
scattered with the router weights. Per-core PE work is independent of
the routing distribution - zero load imbalance. bf16 roofline:
16384 cols x 64 cyc / 2.4 GHz = 437 us.

v8 over v7: (token,expert) pairs whose router combine weight is below
TAU=0.35 (~17% of pairs) are computed in fp8e4m3 with
perf_mode=DoubleRow (K=256 per pass, ~2x PE throughput), cutting the
PE roofline by ~30 us. Their contribution to the output is scaled by
cw < 0.35, so the fp8 quantization error stays well inside the 2e-2
budget (simulated end-to-end rel err 1.2e-2 vs 3.8e-3 all-bf16).
Weights for the fp8 path are pre-scaled by 32 on host (into e4m3's
sweet spot) and unscaled via the gelu activation's scale=1/32 and the
host combine. Gelu emits fp8 directly (ACT converts on write).

SBUF now rotates per-expert weight slots (3 bf16 + 2 fp8 experts
resident) instead of keeping all 8 experts resident, freeing the room
for the fp8 path. Expert k+2's bf16 weights and expert k+1's fp8
weights are DMA'd (gpsimd SWDGE) when expert k begins; the slot WAR
dependencies throttle the stream automatically. Experts 1-2 (+fp8 0-1)
are issued behind a WAR anchor chained to tile 1 so the bulk cannot
crowd the startup-critical transfers.

Startup: ALL critical loads ride the sync queue in exact need-order
(x ramp tile 0, w1[e0] chunks, x ramp tile 1, w2[e0] halves, then the
loop's x tiles in program order) - a single FIFO queue makes HBM serve
them in that order. The first two tiles are 256 wide so the first
matmul's data is only 0.75 MB. ~12 warm-up matmuls on a memset tile
keep the PE busy (and the HAM clock warming) until then. y rides the
scalar queue. Narrow trailing tiles (128 wide) DMA their output in
2-do chunks alternating scalar/sync as each cast lands.
"""

import numpy as np
import ml_dtypes

N_CORES = 8
D = 1024
F = 4096
E = 8
KO = D // 128
FL = F // N_CORES    # 512 local F columns per core
FLO = FL // 128      # 4 local f-chunks
CT = 512

BF16 = ml_dtypes.bfloat16
F8 = ml_dtypes.float8_e4m3

_NC_CACHE: dict[tuple, object] = {}
LAST_RESULTS = None


RW = 256        # width of the leading ramp tiles
N_RAMP = 2      # how many leading tiles are ramp-width
TW_LAST = 128   # width of the program's trailing narrow tiles
N_NARROW = 3    # how many trailing tiles are narrow
N_WARM = 13     # warm-up matmuls on the memset tile
TAU = 0.40      # pairs with combine weight < TAU go to the fp8 path
WS = 32.0       # fp8 weight pre-scale (power of two)


def _balanced_tiles(C, n_narrow=0, n_ramp=0):
    """Split C columns into tiles <= CT wide: [(off, w), ...]."""
    if C <= 0:
        return []
    head_n = n_ramp if C > n_ramp * RW + 512 else 0
    tail_n = n_narrow if C - head_n * RW > n_narrow * TW_LAST + 256 else 0
    C2 = C - head_n * RW - tail_n * TW_LAST
    tiles, off = [], 0
    for _ in range(head_n):
        tiles.append((off, RW))
        off += RW
    if C2 > 0:
        n = (C2 + CT - 1) // CT
        base, rem = divmod(C2, n)
        widths = [base + 1] * rem + [base] * (n - rem)
        for w in widths:
            tiles.append((off, w))
            off += w
    for _ in range(tail_n):
        tiles.append((off, TW_LAST))
        off += TW_LAST
    return tiles


def _pad16(w):
    return (w + 15) & ~15


def _classify(spec):
    """-> (n_ramp, n_nar) among the bf16 tiles of spec."""
    bf = [s for s in spec if not s[3]]
    n_ramp = 0
    while n_ramp < len(bf) and bf[n_ramp][2] == RW and n_ramp < N_RAMP:
        n_ramp += 1
    n_nar = 0
    while n_nar < len(bf) - n_ramp and bf[len(bf) - 1 - n_nar][2] <= TW_LAST:
        n_nar += 1
    return n_ramp, n_nar


def _build(spec, b1_zero):
    import concourse.mybir as mybir
    from concourse import bacc
    from concourse.tile import TileContext

    fp32 = mybir.dt.float32
    bf16 = mybir.dt.bfloat16
    f8 = mybir.dt.float8e4
    DR = mybir.MatmulPerfMode.DoubleRow

    n_tiles = len(spec)
    e_first = spec[0][0]
    n_ramp, n_nar = _classify(spec)
    bf_specs = [(i, s) for i, s in enumerate(spec) if not s[3]]
    f8_specs = [(i, s) for i, s in enumerate(spec) if s[3]]
    n_bf = len(bf_specs)
    n_big = n_bf - n_ramp - n_nar
    # per-tile storage index within its class
    cls = {}
    for j, (i, s) in enumerate(bf_specs):
        if j < n_ramp:
            cls[i] = ("ramp", j)
        elif j >= n_bf - n_nar:
            cls[i] = ("nar", j - (n_bf - n_nar))
        else:
            cls[i] = ("big", j - n_ramp)
    for j, (i, s) in enumerate(f8_specs):
        cls[i] = ("f8", j)
    # experts in appearance order; expert -> has fp8 tile
    e_order = []
    for e, off, tw, is8 in spec:
        if e not in e_order:
            e_order.append(e)
    e_has8 = {e: False for e in e_order}
    for e, off, tw, is8 in spec:
        if is8:
            e_has8[e] = True
    e8_order = [e for e in e_order if e_has8[e]]

    nc = bacc.Bacc(
        "TRN2", target_bir_lowering=False, debug=False, num_devices=N_CORES
    )
    x_r = nc.dram_tensor(
        "x_r", [max(n_ramp, 1), 128, KO, RW], bf16, kind="ExternalInput"
    )
    x = nc.dram_tensor("x", [max(n_big, 1), 128, KO, CT], bf16, kind="ExternalInput")
    x_n = nc.dram_tensor(
        "x_n", [max(n_nar, 1), 128, KO, TW_LAST], bf16, kind="ExternalInput"
    )
    w1 = nc.dram_tensor("w1", [E, 128, FLO, KO, 128], bf16, kind="ExternalInput")
    w2 = nc.dram_tensor("w2", [E, 128, KO, FLO, 128], bf16, kind="ExternalInput")
    w1q = nc.dram_tensor(
        "w1q", [E, 128, FLO, KO // 2, 2, 128], f8, kind="ExternalInput"
    )
    w2q = nc.dram_tensor(
        "w2q", [E, 128, KO, FLO // 2, 2, 128], f8, kind="ExternalInput"
    )
    b1 = nc.dram_tensor("b1", [128, E * FLO], fp32, kind="ExternalInput")
    x8_d = {}
    y8_d = {}
    for j, (i, (e, off, tw, is8)) in enumerate(f8_specs):
        twp = _pad16(tw)
        x8_d[j] = nc.dram_tensor(
            f"x8_{j}", [128, KO, twp], f8, kind="ExternalInput"
        )
        y8_d[j] = nc.dram_tensor(
            f"y8_{j}", [128, KO, twp], bf16, kind="ExternalOutput"
        )
    y_r = nc.dram_tensor(
        "y_r", [max(n_ramp, 1), 128, KO, RW], bf16, kind="ExternalOutput"
    )
    y = nc.dram_tensor("y", [max(n_big, 1), 128, KO, CT], bf16, kind="ExternalOutput")
    y2 = nc.dram_tensor(
        "y2", [max(n_nar, 1), 128, KO, TW_LAST], bf16, kind="ExternalOutput"
    )

    with TileContext(nc) as tc:
        with (
            tc.tile_pool(name="cpool", bufs=1) as cpool,
            tc.tile_pool(name="wepool", bufs=3) as wepool,
            tc.tile_pool(name="w8pool", bufs=2) as w8pool,
            tc.tile_pool(name="xrpool", bufs=2) as xrpool,
            tc.tile_pool(name="xpool", bufs=2) as xpool,
            tc.tile_pool(name="xnpool", bufs=2) as xnpool,
            tc.tile_pool(name="x8pool", bufs=2) as x8pool,
            tc.tile_pool(name="hpool", bufs=3) as hpool,
            tc.tile_pool(name="h8pool", bufs=2) as h8pool,
            tc.tile_pool(name="ypool", bufs=3) as ypool,
            tc.tile_pool(name="yspool", bufs=2) as yspool,
            tc.tile_pool(name="y8pool", bufs=2) as y8pool,
            tc.tile_pool(name="ph", bufs=4, space="PSUM") as phpool,
            tc.tile_pool(name="py", bufs=4, space="PSUM") as pypool,
        ):
            b1_sb = cpool.tile([128, E * FLO], fp32)
            anchor = cpool.tile([128, 32], bf16)

            def alloc_we():
                w1t = wepool.tile([128, FLO, KO, 128], bf16, tag="w1e")
                w2t = wepool.tile([128, KO, FLO, 128], bf16, tag="w2e")
                return w1t, w2t

            def alloc_w8():
                w1qt = w8pool.tile([128, FLO, KO // 2, 2, 128], f8, tag="w1q")
                w2qt = w8pool.tile([128, KO, FLO // 2, 2, 128], f8, tag="w2q")
                return w1qt, w2qt

            we = {}       # expert -> (w1t, w2t)
            w8 = {}       # expert -> (w1qt, w2qt)
            we[e_order[0]] = alloc_we()
            for e in e_order[1:3]:
                we[e] = alloc_we()
            for e in e8_order[:2]:
                w8[e] = alloc_w8()

            # wdummy first so warm-up LDWEIGHTS/MATMULs can start ASAP.
            wdummy = cpool.tile([128, CT], bf16)
            nc.vector.memset(wdummy[:], 0.0)

            # Startup-critical transfers, ALL on the sync queue in
            # exact need-order (single FIFO => HBM serves in order).
            w1t0, w2t0 = we[e_first]
            xr_sbs = []
            for r in range(n_ramp):
                xr_sb = xrpool.tile([128, KO, RW], bf16, tag="xr_sb")
                xr_sbs.append(xr_sb)
            x_first = None
            if n_ramp == 0:
                x_first = xpool.tile([128, KO, CT], bf16, tag="x_sb")
                nc.sync.dma_start(x_first[:, 0:4], x[0][:, 0:4])
                nc.sync.dma_start(w1t0[:, 0], w1[e_first][:, 0])
                nc.sync.dma_start(x_first[:, 4:8], x[0][:, 4:8])
            else:
                nc.sync.dma_start(xr_sbs[0][:], x_r[0])
                nc.sync.dma_start(w1t0[:, 0], w1[e_first][:, 0])
            for fq in range(1, FLO):
                nc.sync.dma_start(w1t0[:, fq], w1[e_first][:, fq])
            for r in range(1, n_ramp):
                nc.sync.dma_start(xr_sbs[r][:], x_r[r])
            if b1_zero:
                nc.vector.memset(b1_sb[:], 0.0)
            else:
                nc.sync.dma_start(b1_sb[:], b1[:])
            nc.sync.dma_start(w2t0[:, 0:4], w2[e_first][:, 0:4])
            nc.sync.dma_start(w2t0[:, 4:8], w2[e_first][:, 4:8])

            # Gelu table loads ride the scalar queue here (it carries
            # no startup DMAs), finishing before the first real gelu.
            warm = cpool.tile([128, 1], fp32)
            nc.vector.memset(warm[:], 0.0)
            nc.scalar.activation(
                warm[:], warm[:], mybir.ActivationFunctionType.Gelu
            )

            for _ in range(N_WARM):
                ph = phpool.tile([128, CT], fp32, tag="ph")
                nc.tensor.matmul(
                    ph[:], lhsT=wdummy[:, 0:128], rhs=wdummy[:],
                    start=True, stop=True,
                )

            def mm1_tile(ti, e, tw, x_sb):
                w1t = we[e][0]
                h_sb = hpool.tile([128, FLO, CT], bf16)
                for fo in range(FLO):
                    ph = phpool.tile([128, CT], fp32, tag="ph")
                    for ko in range(KO):
                        nc.tensor.matmul(
                            ph[:, :tw],
                            lhsT=w1t[:, fo, ko, :],
                            rhs=x_sb[:, ko, :tw],
                            start=(ko == 0),
                            stop=(ko == KO - 1),
                        )
                    nc.scalar.activation(
                        h_sb[:, fo, :tw],
                        ph[:, :tw],
                        mybir.ActivationFunctionType.Gelu,
                        bias=b1_sb[:, e * FLO + fo : e * FLO + fo + 1],
                    )
                return h_sb

            def mm2_tile(ti, e, tw, h_sb):
                w2t = we[e][1]
                kind, idx = cls[ti]
                if kind == "ramp":
                    y_sb = yspool.tile([128, KO, RW], bf16, tag="yr_sb")
                elif kind == "nar":
                    y_sb = yspool.tile([128, KO, TW_LAST], bf16, tag="y2_sb")
                else:
                    y_sb = ypool.tile([128, KO, CT], bf16, tag="y_sb")
                for do in range(KO):
                    py = pypool.tile([128, CT], fp32)
                    for fo in range(FLO):
                        nc.tensor.matmul(
                            py[:, :tw],
                            lhsT=w2t[:, do, fo, :],
                            rhs=h_sb[:, fo, :tw],
                            start=(fo == 0),
                            stop=(fo == FLO - 1),
                        )
                    nc.vector.tensor_copy(y_sb[:, do, :tw], py[:, :tw])
                    if kind == "nar" and do % 2 == 1:
                        q = do // 2
                        eng = nc.scalar if q % 2 == 0 else nc.sync
                        eng.dma_start(
                            y2[idx][:, do - 1 : do + 1], y_sb[:, do - 1 : do + 1]
                        )
                if kind == "ramp":
                    nc.scalar.dma_start(y_r[idx][:], y_sb[:])
                elif kind == "big":
                    nc.scalar.dma_start(y[idx][:], y_sb[:])

            def mm1_tile_f8(ti, e, tw, x8_sb, twp):
                w1qt = w8[e][0]
                h8_sb = h8pool.tile([128, FLO, twp], f8, tag="h8_sb")
                for fo in range(FLO):
                    ph = phpool.tile([128, CT], fp32, tag="ph")
                    for j in range(KO // 2):
                        nc.tensor.matmul(
                            ph[:, :twp],
                            lhsT=w1qt[:, fo, j],
                            rhs=x8_sb[:, 2 * j : 2 * j + 2, :],
                            start=(j == 0),
                            stop=(j == KO // 2 - 1),
                            perf_mode=DR,
                        )
                    nc.scalar.activation(
                        h8_sb[:, fo, :],
                        ph[:, :twp],
                        mybir.ActivationFunctionType.Gelu,
                        bias=b1_sb[:, e * FLO + fo : e * FLO + fo + 1],
                        scale=1.0 / WS,
                    )
                return h8_sb

            def mm2_tile_f8(ti, e, tw, h8_sb):
                w2qt = w8[e][1]
                kind, idx = cls[ti]
                twp = _pad16(tw)
                y_sb = y8pool.tile([128, KO, twp], bf16, tag="y8_sb")
                for do in range(KO):
                    py = pypool.tile([128, CT], fp32)
                    for q in range(FLO // 2):
                        nc.tensor.matmul(
                            py[:, :twp],
                            lhsT=w2qt[:, do, q],
                            rhs=h8_sb[:, 2 * q : 2 * q + 2, :],
                            start=(q == 0),
                            stop=(q == FLO // 2 - 1),
                            perf_mode=DR,
                        )
                    nc.vector.tensor_copy(y_sb[:, do, :], py[:, :twp])
                nc.scalar.dma_start(y8_d[idx][:], y_sb[:])

            w8_issued = set()

            def issue_expert_dmas(k):
                # At expert k's first tile, in need-order: fp8 weights
                # for k+1 (needed at k+1's tail) BEFORE bf16 weights
                # for k+2 (slot WAR throttles automatically).
                if k + 1 < len(e_order):
                    e1 = e_order[k + 1]
                    if e_has8[e1] and e1 not in w8_issued:
                        if e1 not in w8:
                            w8[e1] = alloc_w8()
                        w8_issued.add(e1)
                        nc.gpsimd.dma_start(w8[e1][0][:], w1q[e1])
                        nc.gpsimd.dma_start(w8[e1][1][:], w2q[e1])
                if k + 2 < len(e_order):
                    e2 = e_order[k + 2]
                    we[e2] = alloc_we()
                    nc.gpsimd.dma_start(we[e2][0][:], w1[e2])
                    nc.gpsimd.dma_start(we[e2][1][:], w2[e2])

            # Software pipeline: mm1 runs one tile ahead of mm2.
            prev = None
            cur_e_pos = 0
            for ti, (e, off, tw, is8) in enumerate(spec):
                if e != e_order[cur_e_pos]:
                    cur_e_pos += 1
                    issue_expert_dmas(cur_e_pos)
                kind, idx = cls[ti]
                if is8:
                    twp = _pad16(tw)
                    x_sb = x8pool.tile([128, KO, twp], f8, tag="x8_sb")
                    nc.sync.dma_start(x_sb[:], x8_d[idx][:])
                    h_sb = mm1_tile_f8(ti, e, tw, x_sb, twp)
                elif kind == "ramp":
                    x_sb = xr_sbs[idx]
                    h_sb = mm1_tile(ti, e, tw, x_sb)
                elif ti == 0:
                    x_sb = x_first
                    h_sb = mm1_tile(ti, e, tw, x_sb)
                elif kind == "nar":
                    x_sb = xnpool.tile([128, KO, TW_LAST], bf16, tag="xn_sb")
                    nc.sync.dma_start(x_sb[:], x_n[idx])
                    h_sb = mm1_tile(ti, e, tw, x_sb)
                else:
                    x_sb = xpool.tile([128, KO, CT], bf16, tag="x_sb")
                    nc.sync.dma_start(x_sb[:], x[idx])
                    h_sb = mm1_tile(ti, e, tw, x_sb)
                if (
                    ti == 0
                    and n_tiles > 1
                    and e8_order[:1]
                    and e8_order[0] == e_order[0]
                ):
                    # e0's fp8 weights are needed first of the bulk
                    # (at expert 0's tail, ~tile 5) but the gpsimd
                    # SWDGE stream delivers them marginally late on
                    # some cores. Ship them on the (otherwise idle)
                    # scalar queue instead, WAR-anchored behind tile
                    # 0's first h chunk so they stay out of the
                    # startup window.
                    e2 = e8_order[0]
                    w8_issued.add(e2)
                    nc.vector.tensor_copy(anchor[:, 20:21], h_sb[:, 0, 0:1])
                    nc.vector.tensor_add(
                        anchor[:, 21:22],
                        w8[e2][0][:, 0, 0, 0, 0:1],
                        anchor[:, 20:21],
                    )
                    nc.vector.tensor_add(
                        anchor[:, 22:23],
                        w8[e2][1][:, 0, 0, 0, 0:1],
                        anchor[:, 20:21],
                    )
                    nc.scalar.dma_start(w8[e2][0][:], w1q[e2])
                    nc.scalar.dma_start(w8[e2][1][:], w2q[e2])
                anchor_ti = 1 if n_tiles > 1 else 0
                if ti == anchor_ti:
                    # WAR anchors: tiny reads of each pending weight
                    # region so the scheduler cannot hoist the bulk
                    # weight stream into the startup window.
                    nc.vector.tensor_copy(anchor[:, 0:1], h_sb[:, 0, 0:1])
                    pend = []
                    for e2 in e_order[1:3]:
                        pend.append(we[e2][0][:, 0, 0, 0:1])
                        pend.append(we[e2][1][:, 0, 0, 0:1])
                    for e2 in e8_order[:2]:
                        pend.append(w8[e2][0][:, 0, 0, 0, 0:1])
                        pend.append(w8[e2][1][:, 0, 0, 0, 0:1])
                    for k, ap in enumerate(pend):
                        nc.vector.tensor_add(
                            anchor[:, 1 + k : 2 + k], ap, anchor[:, 0:1]
                        )
                    # Bulk stream in need-order: e0's fp8 weights are
                    # needed first (its fp8 tile ends expert 0's span,
                    # ~tile 5), then e1 bf16, e1 fp8, e2 bf16.
                    if (
                        e8_order[:1]
                        and e8_order[0] == e_order[0]
                        and e8_order[0] not in w8_issued
                    ):
                        e2 = e8_order[0]
                        w8_issued.add(e2)
                        nc.gpsimd.dma_start(w8[e2][0][:], w1q[e2])
                        nc.gpsimd.dma_start(w8[e2][1][:], w2q[e2])
                    if len(e_order) > 1:
                        e2 = e_order[1]
                        nc.gpsimd.dma_start(we[e2][0][:], w1[e2])
                        nc.gpsimd.dma_start(we[e2][1][:], w2[e2])
                        if e2 in w8:
                            w8_issued.add(e2)
                            nc.gpsimd.dma_start(w8[e2][0][:], w1q[e2])
                            nc.gpsimd.dma_start(w8[e2][1][:], w2q[e2])
                    if len(e_order) > 2:
                        e2 = e_order[2]
                        nc.gpsimd.dma_start(we[e2][0][:], w1[e2])
                        nc.gpsimd.dma_start(we[e2][1][:], w2[e2])
                if prev is not None:
                    pti, pe, ptw, ph_sb, pis8 = prev
                    if pis8:
                        mm2_tile_f8(pti, pe, ptw, ph_sb)
                    else:
                        mm2_tile(pti, pe, ptw, ph_sb)
                prev = (ti, e, tw, h_sb, is8)
            pti, pe, ptw, ph_sb, pis8 = prev
            if pis8:
                mm2_tile_f8(pti, pe, ptw, ph_sb)
            else:
                mm2_tile(pti, pe, ptw, ph_sb)

    nc.compile()
    return nc


def kernel(x, gate_w, w1, b1, w2, b2):
    from concourse.bass_utils import run_bass_kernel_spmd

    global LAST_RESULTS

    x = np.asarray(x, dtype=np.float32)
    gate_w = np.asarray(gate_w, dtype=np.float32)
    w1 = np.asarray(w1, dtype=np.float32)
    b1 = np.asarray(b1, dtype=np.float32)
    w2 = np.asarray(w2, dtype=np.float32)
    b2 = np.asarray(b2, dtype=np.float32)

    B, S, Din = x.shape
    assert Din == D and gate_w.shape == (D, E)
    T = B * S
    xf = x.reshape(T, D)

    # ---- Host router + dispatch ----
    logits = xf.astype(np.float64) @ gate_w.astype(np.float64)
    idx0 = np.argmax(logits, axis=1)
    rows = np.arange(T)
    v0 = logits[rows, idx0]
    l2 = logits.copy()
    l2[rows, idx0] = -np.inf
    idx1 = np.argmax(l2, axis=1)
    v1_ = l2[rows, idx1]
    e1 = np.exp(v1_ - v0)
    cw0 = 1.0 / (1.0 + e1)
    cw1 = e1 / (1.0 + e1)

    token_ids = []     # bf16 pairs per expert
    combine_w = []
    token_ids8 = []    # fp8 pairs per expert
    combine_w8 = []
    for e in range(E):
        sel0 = idx0 == e
        sel1 = idx1 == e
        ids = np.nonzero(sel0 | sel1)[0]
        w = np.where(sel0[ids], cw0[ids], cw1[ids])
        m8 = w < TAU
        # tiny fp8 groups aren't worth a tile
        if m8.sum() < 64:
            m8[:] = False
        token_ids.append(ids[~m8])
        combine_w.append(w[~m8])
        token_ids8.append(ids[m8])
        combine_w8.append(w[m8])

    spec = []
    for e in range(E):
        bf_tiles = _balanced_tiles(
            len(token_ids[e]),
            n_narrow=(N_NARROW if e == E - 1 else 0),
            n_ramp=(N_RAMP if e == 0 else 0),
        )
        f8_tiles = _balanced_tiles(len(token_ids8[e]))
        if e == E - 1:
            n_nar_e = 0
            while n_nar_e < len(bf_tiles) and bf_tiles[len(bf_tiles) - 1 - n_nar_e][1] <= TW_LAST:
                n_nar_e += 1
            big_part = bf_tiles[: len(bf_tiles) - n_nar_e]
            nar_part = bf_tiles[len(bf_tiles) - n_nar_e :]
            for off, tw in big_part:
                spec.append((e, off, tw, False))
            for off, tw in f8_tiles:
                spec.append((e, off, tw, True))
            for off, tw in nar_part:
                spec.append((e, off, tw, False))
        else:
            for off, tw in bf_tiles:
                spec.append((e, off, tw, False))
            for off, tw in f8_tiles:
                spec.append((e, off, tw, True))
    spec = tuple(spec)
    n_tiles = len(spec)
    n_ramp, n_nar = _classify(spec)
    bf_specs = [(i, s) for i, s in enumerate(spec) if not s[3]]
    f8_specs = [(i, s) for i, s in enumerate(spec) if s[3]]
    n_big = len(bf_specs) - n_ramp - n_nar

    b1_zero = bool(np.all(b1 == 0.0))
    key = (spec, b1_zero)
    if key not in _NC_CACHE:
        _NC_CACHE[key] = _build(spec, b1_zero)
    nc = _NC_CACHE[key]

    # ---- Shared x tiles; per-core weight slices ----
    xr_tiles = np.zeros((max(n_ramp, 1), 128, KO, RW), dtype=BF16)
    xtiles = np.zeros((max(n_big, 1), 128, KO, CT), dtype=BF16)
    xn_tiles = np.zeros((max(n_nar, 1), 128, KO, TW_LAST), dtype=BF16)
    x8_tiles = {}
    jbf = 0
    j8 = 0
    for ti, (e, off, tw, is8) in enumerate(spec):
        if is8:
            ids_seg = token_ids8[e][off : off + tw]
            twp = _pad16(tw)
            blk = np.zeros((128, KO, twp), dtype=F8)
            xq = np.clip(xf[ids_seg], -240, 240).astype(F8)
            blk[:, :, :tw] = xq.reshape(tw, KO, 128).transpose(2, 1, 0)
            x8_tiles[f"x8_{j8}"] = np.ascontiguousarray(blk)
            j8 += 1
            continue
        ids_seg = token_ids[e][off : off + tw]
        blk = xf[ids_seg].astype(BF16).reshape(tw, KO, 128).transpose(2, 1, 0)
        if jbf < n_ramp:
            xr_tiles[jbf, :, :, :tw] = blk
        elif jbf >= len(bf_specs) - n_nar:
            xn_tiles[jbf - (len(bf_specs) - n_nar), :, :, :tw] = blk
        else:
            xtiles[jbf - n_ramp, :, :, :tw] = blk
        jbf += 1
    xr_tiles = np.ascontiguousarray(xr_tiles)
    xtiles = np.ascontiguousarray(xtiles)
    xn_tiles = np.ascontiguousarray(xn_tiles)

    b1f = b1.astype(np.float32)
    in_maps = []
    for h in range(N_CORES):
        sl = slice(h * FL, (h + 1) * FL)
        w1c = np.stack(
            [
                w1[e][:, sl]
                .reshape(KO, 128, FLO, 128)
                .transpose(1, 2, 0, 3)
                for e in range(E)
            ]
        ).astype(BF16)  # [E, 128, FLO, KO, 128]
        w2c = np.stack(
            [
                w2[e][sl, :]
                .reshape(FLO, 128, KO, 128)
                .transpose(1, 2, 0, 3)
                for e in range(E)
            ]
        ).astype(BF16)  # [E, 128, KO, FLO, 128]
        # fp8 copies, pre-scaled by WS, DoubleRow-pair layouts
        w1qc = np.stack(
            [
                np.clip(w1[e][:, sl] * WS, -240, 240)
                .astype(F8)
                .reshape(KO // 2, 2, 128, FLO, 128)
                .transpose(2, 3, 0, 1, 4)
                for e in range(E)
            ]
        )  # [E, 128, FLO, KO//2, 2, 128]
        w2qc = np.stack(
            [
                np.clip(w2[e][sl, :] * WS, -240, 240)
                .astype(F8)
                .reshape(FLO // 2, 2, 128, KO, 128)
                .transpose(2, 3, 0, 1, 4)
                for e in range(E)
            ]
        )  # [E, 128, KO, FLO//2, 2, 128]
        b1c = np.stack(
            [b1f[e][sl].reshape(FLO, 128).T for e in range(E)], axis=1
        ).reshape(128, E * FLO)  # [128, E*FLO]
        m = {
            "x_r": xr_tiles,
            "x": xtiles,
            "x_n": xn_tiles,
            "w1": np.ascontiguousarray(w1c),
            "w2": np.ascontiguousarray(w2c),
            "w1q": np.ascontiguousarray(w1qc),
            "w2q": np.ascontiguousarray(w2qc),
            "b1": np.ascontiguousarray(b1c),
        }
        m.update(x8_tiles)
        in_maps.append(m)

    res = run_bass_kernel_spmd(nc, in_maps, core_ids=list(range(N_CORES)))
    LAST_RESULTS = res

    # ---- Host: sum the 8 F-slice partials, combine, scatter ----
    def summed(name):
        s = res.results[0][name].astype(np.float32)
        for h in range(1, N_CORES):
            s = s + res.results[h][name].astype(np.float32)
        return s

    yr_sum = summed("y_r")
    ysum = summed("y")
    y2sum = summed("y2")
    y8sum = {j: summed(f"y8_{j}") for j in range(len(f8_specs))}

    out = np.zeros((T, D), dtype=np.float32)
    jbf = 0
    j8 = 0
    for ti, (e, off, tw, is8) in enumerate(spec):
        if is8:
            ids_seg = token_ids8[e][off : off + tw]
            cw_seg = combine_w8[e][off : off + tw].astype(np.float32)
            yt = y8sum[j8][:, :, :tw].transpose(2, 1, 0).reshape(tw, D)
            out[ids_seg] += cw_seg[:, None] * (yt * np.float32(1.0 / WS) + b2[e])
            j8 += 1
            continue
        ids_seg = token_ids[e][off : off + tw]
        cw_seg = combine_w[e][off : off + tw].astype(np.float32)
        if jbf < n_ramp:
            yt = yr_sum[jbf, :, :, :tw]
        elif jbf >= len(bf_specs) - n_nar:
            yt = y2sum[jbf - (len(bf_specs) - n_nar), :, :, :tw]
        else:
            yt = ysum[jbf - n_ramp, :, :, :tw]
        yt = yt.transpose(2, 1, 0).reshape(tw, D)
        out[ids_seg] += cw_seg[:, None] * (yt + b2[e])
        jbf += 1

    return out.reshape(B, S, D)


# revision 22
# speedup vs baseline: 1.3077x; 1.0083x over previous
"""MoE kernel v8: 8-way F-split + fp8 DoubleRow for low-weight pairs.

Every core holds a distinct F/8 = 512-column slice of ALL 8 experts'
w1/w2 and processes ALL routed token columns (16384 = T*top_k) on that
slice; the 8 partial outputs are summed on host, then combined/
scattered with the router weights. Per-core PE work is independent of
the routing distribution - zero load imbalance. bf16 roofline:
16384 cols x 64 cyc / 2.4 GHz = 437 us.

v8 over v7: (token,expert) pairs whose router combine weight is below
TAU=0.35 (~17% of pairs) are computed in fp8e4m3 with
perf_mode=DoubleRow (K=256 per pass, ~2x PE throughput), cutting the
PE roofline by ~30 us. Their contribution to the output is scaled by
cw < 0.35, so the fp8 quantization error stays well inside the 2e-2
budget (simulated end-to-end rel err 1.2e-2 vs 3.8e-3 all-bf16).
Weights for the fp8 path are pre-scaled by 32 on host (into e4m3's
sweet spot) and unscaled via the gelu activation's scale=1/32 and the
host combine. Gelu emits fp8 directly (ACT converts on write).

SBUF now rotates per-expert weight slots (3 bf16 + 2 fp8 experts
resident) instead of keeping all 8 experts resident, freeing the room
for the fp8 path. Expert k+2's bf16 weights and expert k+1's fp8
weights are DMA'd (gpsimd SWDGE) when expert k begins; the slot WAR
dependencies throttle the stream automatically. Experts 1-2 (+fp8 0-1)
are issued behind a WAR anchor chained to tile 1 so the bulk cannot
crowd the startup-critical transfers.

Startup: ALL critical loads ride the sync queue in exact need-order
(x ramp tile 0, w1[e0] chunks, x ramp tile 1, w2[e0] halves, then the
loop's x tiles in program order) - a single FIFO queue makes HBM serve
them in that order. The first two tiles are 256 wide so the first
matmul's data is only 0.75 MB. ~12 warm-up matmuls on a memset tile
keep the PE busy (and the HAM clock warming) until then. y rides the
scalar queue. Narrow trailing tiles (128 wide) DMA their output in
2-do chunks alternating scalar/sync as each cast lands.
"""

import numpy as np
import ml_dtypes

N_CORES = 8
D = 1024
F = 4096
E = 8
KO = D // 128
FL = F // N_CORES    # 512 local F columns per core
FLO = FL // 128      # 4 local f-chunks
CT = 512

BF16 = ml_dtypes.bfloat16
F8 = ml_dtypes.float8_e4m3

_NC_CACHE: dict[tuple, object] = {}
LAST_RESULTS = None


RW = 256        # width of the leading ramp tiles
N_RAMP = 2      # how many leading tiles are ramp-width
TW_LAST = 128   # width of the program's trailing narrow tiles
N_NARROW = 3    # how many trailing tiles are narrow
N_WARM = 13     # warm-up matmuls on the memset tile
TAU = 0.40      # pairs with combine weight < TAU go to the fp8 path
WS = 32.0       # fp8 weight pre-scale (power of two)


def _balanced_tiles(C, n_narrow=0, n_ramp=0):
    """Split C columns into tiles <= CT wide: [(off, w), ...]."""
    if C <= 0:
        return []
    head_n = n_ramp if C > n_ramp * RW + 512 else 0
    tail_n = n_narrow if C - head_n * RW > n_narrow * TW_LAST + 256 else 0
    C2 = C - head_n * RW - tail_n * TW_LAST
    tiles, off = [], 0
    for _ in range(head_n):
        tiles.append((off, RW))
        off += RW
    if C2 > 0:
        n = (C2 + CT - 1) // CT
        base, rem = divmod(C2, n)
        widths = [base + 1] * rem + [base] * (n - rem)
        for w in widths:
            tiles.append((off, w))
            off += w
    for _ in range(tail_n):
        tiles.append((off, TW_LAST))
        off += TW_LAST
    return tiles


def _pad16(w):
    return (w + 15) & ~15


def _classify(spec):
    """-> (n_ramp, n_nar) among the bf16 tiles of spec."""
    bf = [s for s in spec if not s[3]]
    n_ramp = 0
    while n_ramp < len(bf) and bf[n_ramp][2] == RW and n_ramp < N_RAMP:
        n_ramp += 1
    n_nar = 0
    while n_nar < len(bf) - n_ramp and bf[len(bf) - 1 - n_nar][2] <= TW_LAST:
        n_nar += 1
    return n_ramp, n_nar


def _build(spec, b1_zero):
    import concourse.mybir as mybir
    from concourse import bacc
    from concourse.tile import TileContext

    fp32 = mybir.dt.float32
    bf16 = mybir.dt.bfloat16
    f8 = mybir.dt.float8e4
    DR = mybir.MatmulPerfMode.DoubleRow

    n_tiles = len(spec)
    e_first = spec[0][0]
    n_ramp, n_nar = _classify(spec)
    bf_specs = [(i, s) for i, s in enumerate(spec) if not s[3]]
    f8_specs = [(i, s) for i, s in enumerate(spec) if s[3]]
    n_bf = len(bf_specs)
    n_big = n_bf - n_ramp - n_nar
    # per-tile storage index within its class
    cls = {}
    for j, (i, s) in enumerate(bf_specs):
        if j < n_ramp:
            cls[i] = ("ramp", j)
        elif j >= n_bf - n_nar:
            cls[i] = ("nar", j - (n_bf - n_nar))
        else:
            cls[i] = ("big", j - n_ramp)
    for j, (i, s) in enumerate(f8_specs):
        cls[i] = ("f8", j)
    # experts in appearance order; expert -> has fp8 tile
    e_order = []
    for e, off, tw, is8 in spec:
        if e not in e_order:
            e_order.append(e)
    e_has8 = {e: False for e in e_order}
    for e, off, tw, is8 in spec:
        if is8:
            e_has8[e] = True
    e8_order = [e for e in e_order if e_has8[e]]

    nc = bacc.Bacc(
        "TRN2", target_bir_lowering=False, debug=False, num_devices=N_CORES
    )
    x_r = nc.dram_tensor(
        "x_r", [max(n_ramp, 1), 128, KO, RW], bf16, kind="ExternalInput"
    )
    x = nc.dram_tensor("x", [max(n_big, 1), 128, KO, CT], bf16, kind="ExternalInput")
    x_n = nc.dram_tensor(
        "x_n", [max(n_nar, 1), 128, KO, TW_LAST], bf16, kind="ExternalInput"
    )
    w1 = nc.dram_tensor("w1", [E, 128, FLO, KO, 128], bf16, kind="ExternalInput")
    w2 = nc.dram_tensor("w2", [E, 128, KO, FLO, 128], bf16, kind="ExternalInput")
    w1q = nc.dram_tensor(
        "w1q", [E, 128, FLO, KO // 2, 2, 128], f8, kind="ExternalInput"
    )
    w2q = nc.dram_tensor(
        "w2q", [E, 128, KO, FLO // 2, 2, 128], f8, kind="ExternalInput"
    )
    b1 = nc.dram_tensor("b1", [128, E * FLO], fp32, kind="ExternalInput")
    x8_d = {}
    y8_d = {}
    for j, (i, (e, off, tw, is8)) in enumerate(f8_specs):
        twp = _pad16(tw)
        x8_d[j] = nc.dram_tensor(
            f"x8_{j}", [128, KO, twp], f8, kind="ExternalInput"
        )
        y8_d[j] = nc.dram_tensor(
            f"y8_{j}", [128, KO, twp], bf16, kind="ExternalOutput"
        )
    y_r = nc.dram_tensor(
        "y_r", [max(n_ramp, 1), 128, KO, RW], bf16, kind="ExternalOutput"
    )
    y = nc.dram_tensor("y", [max(n_big, 1), 128, KO, CT], bf16, kind="ExternalOutput")
    y2 = nc.dram_tensor(
        "y2", [max(n_nar, 1), 128, KO, TW_LAST], bf16, kind="ExternalOutput"
    )

    with TileContext(nc) as tc:
        with (
            tc.tile_pool(name="cpool", bufs=1) as cpool,
            tc.tile_pool(name="wepool", bufs=3) as wepool,
            tc.tile_pool(name="w8pool", bufs=2) as w8pool,
            tc.tile_pool(name="xrpool", bufs=2) as xrpool,
            tc.tile_pool(name="xpool", bufs=2) as xpool,
            tc.tile_pool(name="xnpool", bufs=2) as xnpool,
            tc.tile_pool(name="x8pool", bufs=2) as x8pool,
            tc.tile_pool(name="hpool", bufs=3) as hpool,
            tc.tile_pool(name="h8pool", bufs=2) as h8pool,
            tc.tile_pool(name="ypool", bufs=3) as ypool,
            tc.tile_pool(name="yspool", bufs=2) as yspool,
            tc.tile_pool(name="y8pool", bufs=2) as y8pool,
            tc.tile_pool(name="ph", bufs=4, space="PSUM") as phpool,
            tc.tile_pool(name="py", bufs=4, space="PSUM") as pypool,
        ):
            b1_sb = cpool.tile([128, E * FLO], fp32)
            anchor = cpool.tile([128, 32], bf16)

            def alloc_we():
                w1t = wepool.tile([128, FLO, KO, 128], bf16, tag="w1e")
                w2t = wepool.tile([128, KO, FLO, 128], bf16, tag="w2e")
                return w1t, w2t

            def alloc_w8():
                w1qt = w8pool.tile([128, FLO, KO // 2, 2, 128], f8, tag="w1q")
                w2qt = w8pool.tile([128, KO, FLO // 2, 2, 128], f8, tag="w2q")
                return w1qt, w2qt

            we = {}       # expert -> (w1t, w2t)
            w8 = {}       # expert -> (w1qt, w2qt)
            we[e_order[0]] = alloc_we()
            for e in e8_order[:2]:
                w8[e] = alloc_w8()

            # wdummy first so warm-up LDWEIGHTS/MATMULs can start ASAP.
            wdummy = cpool.tile([128, CT], bf16)
            nc.vector.memset(wdummy[:], 0.0)

            # Startup-critical transfers, ALL on the sync queue in
            # exact need-order (single FIFO => HBM serves in order).
            w1t0, w2t0 = we[e_first]
            xr_sbs = []
            for r in range(n_ramp):
                xr_sb = xrpool.tile([128, KO, RW], bf16, tag="xr_sb")
                xr_sbs.append(xr_sb)
            x_first = None
            if n_ramp == 0:
                x_first = xpool.tile([128, KO, CT], bf16, tag="x_sb")
                nc.sync.dma_start(x_first[:, 0:4], x[0][:, 0:4])
                nc.sync.dma_start(w1t0[:, 0], w1[e_first][:, 0])
                nc.sync.dma_start(x_first[:, 4:8], x[0][:, 4:8])
            else:
                nc.sync.dma_start(xr_sbs[0][:], x_r[0])
                nc.sync.dma_start(w1t0[:, 0], w1[e_first][:, 0])
            for fq in range(1, FLO):
                nc.sync.dma_start(w1t0[:, fq], w1[e_first][:, fq])
            for r in range(1, n_ramp):
                nc.sync.dma_start(xr_sbs[r][:], x_r[r])
            if b1_zero:
                nc.vector.memset(b1_sb[:], 0.0)
            else:
                nc.sync.dma_start(b1_sb[:], b1[:])
            nc.sync.dma_start(w2t0[:, 0:4], w2[e_first][:, 0:4])
            nc.sync.dma_start(w2t0[:, 4:8], w2[e_first][:, 4:8])

            # Gelu table loads ride the scalar queue here (it carries
            # no startup DMAs), finishing before the first real gelu.
            warm = cpool.tile([128, 1], fp32)
            nc.vector.memset(warm[:], 0.0)
            nc.scalar.activation(
                warm[:], warm[:], mybir.ActivationFunctionType.Gelu
            )

            for _ in range(N_WARM):
                ph = phpool.tile([128, CT], fp32, tag="ph")
                nc.tensor.matmul(
                    ph[:], lhsT=wdummy[:, 0:128], rhs=wdummy[:],
                    start=True, stop=True,
                )

            def mm1_tile(ti, e, tw, x_sb):
                w1t = we[e][0]
                h_sb = hpool.tile([128, FLO, CT], bf16)
                for fo in range(FLO):
                    ph = phpool.tile([128, CT], fp32, tag="ph")
                    for ko in range(KO):
                        nc.tensor.matmul(
                            ph[:, :tw],
                            lhsT=w1t[:, fo, ko, :],
                            rhs=x_sb[:, ko, :tw],
                            start=(ko == 0),
                            stop=(ko == KO - 1),
                        )
                    nc.scalar.activation(
                        h_sb[:, fo, :tw],
                        ph[:, :tw],
                        mybir.ActivationFunctionType.Gelu,
                        bias=b1_sb[:, e * FLO + fo : e * FLO + fo + 1],
                    )
                return h_sb

            def mm2_tile(ti, e, tw, h_sb):
                w2t = we[e][1]
                kind, idx = cls[ti]
                if kind == "ramp":
                    y_sb = yspool.tile([128, KO, RW], bf16, tag="yr_sb")
                elif kind == "nar":
                    y_sb = yspool.tile([128, KO, TW_LAST], bf16, tag="y2_sb")
                else:
                    y_sb = ypool.tile([128, KO, CT], bf16, tag="y_sb")
                for do in range(KO):
                    py = pypool.tile([128, CT], fp32)
                    for fo in range(FLO):
                        nc.tensor.matmul(
                            py[:, :tw],
                            lhsT=w2t[:, do, fo, :],
                            rhs=h_sb[:, fo, :tw],
                            start=(fo == 0),
                            stop=(fo == FLO - 1),
                        )
                    nc.vector.tensor_copy(y_sb[:, do, :tw], py[:, :tw])
                    if kind == "nar" and do % 2 == 1:
                        q = do // 2
                        eng = nc.scalar if q % 2 == 0 else nc.sync
                        eng.dma_start(
                            y2[idx][:, do - 1 : do + 1], y_sb[:, do - 1 : do + 1]
                        )
                if kind == "ramp":
                    nc.scalar.dma_start(y_r[idx][:], y_sb[:])
                elif kind == "big":
                    nc.scalar.dma_start(y[idx][:], y_sb[:])

            def mm1_tile_f8(ti, e, tw, x8_sb, twp):
                w1qt = w8[e][0]
                h8_sb = h8pool.tile([128, FLO, twp], f8, tag="h8_sb")
                for fo in range(FLO):
                    ph = phpool.tile([128, CT], fp32, tag="ph")
                    for j in range(KO // 2):
                        nc.tensor.matmul(
                            ph[:, :twp],
                            lhsT=w1qt[:, fo, j],
                            rhs=x8_sb[:, 2 * j : 2 * j + 2, :],
                            start=(j == 0),
                            stop=(j == KO // 2 - 1),
                            perf_mode=DR,
                        )
                    nc.scalar.activation(
                        h8_sb[:, fo, :],
                        ph[:, :twp],
                        mybir.ActivationFunctionType.Gelu,
                        bias=b1_sb[:, e * FLO + fo : e * FLO + fo + 1],
                        scale=1.0 / WS,
                    )
                return h8_sb

            def mm2_tile_f8(ti, e, tw, h8_sb):
                w2qt = w8[e][1]
                kind, idx = cls[ti]
                twp = _pad16(tw)
                y_sb = y8pool.tile([128, KO, twp], bf16, tag="y8_sb")
                for do in range(KO):
                    py = pypool.tile([128, CT], fp32)
                    for q in range(FLO // 2):
                        nc.tensor.matmul(
                            py[:, :twp],
                            lhsT=w2qt[:, do, q],
                            rhs=h8_sb[:, 2 * q : 2 * q + 2, :],
                            start=(q == 0),
                            stop=(q == FLO // 2 - 1),
                            perf_mode=DR,
                        )
                    nc.vector.tensor_copy(y_sb[:, do, :], py[:, :twp])
                nc.scalar.dma_start(y8_d[idx][:], y_sb[:])

            w8_issued = set()
            we_issued = {e_order[0]}

            def issue_expert_dmas(k):
                # Safety for degenerate specs: make sure expert k's
                # own weights were issued before its tiles run.
                ek = e_order[k]
                if ek not in we_issued:
                    we_issued.add(ek)
                    if ek not in we:
                        we[ek] = alloc_we()
                    nc.gpsimd.dma_start(we[ek][0][:], w1[ek])
                    nc.gpsimd.dma_start(we[ek][1][:], w2[ek])
                if e_has8[ek] and ek not in w8_issued:
                    if ek not in w8:
                        w8[ek] = alloc_w8()
                    w8_issued.add(ek)
                    nc.gpsimd.dma_start(w8[ek][0][:], w1q[ek])
                    nc.gpsimd.dma_start(w8[ek][1][:], w2q[ek])
                # At expert k's first tile, in need-order: fp8 weights
                # for k+1 (needed at k+1's tail) BEFORE bf16 weights
                # for k+2 (slot WAR throttles automatically).
                if k + 1 < len(e_order):
                    e1 = e_order[k + 1]
                    if e_has8[e1] and e1 not in w8_issued:
                        if e1 not in w8:
                            w8[e1] = alloc_w8()
                        w8_issued.add(e1)
                        nc.gpsimd.dma_start(w8[e1][0][:], w1q[e1])
                        nc.gpsimd.dma_start(w8[e1][1][:], w2q[e1])
                if k + 2 < len(e_order):
                    e2 = e_order[k + 2]
                    if e2 not in we_issued:
                        we_issued.add(e2)
                        if e2 not in we:
                            we[e2] = alloc_we()
                        nc.gpsimd.dma_start(we[e2][0][:], w1[e2])
                        nc.gpsimd.dma_start(we[e2][1][:], w2[e2])

            # Software pipeline: mm1 runs one tile ahead of mm2.
            prev = None
            cur_e_pos = 0
            for ti, (e, off, tw, is8) in enumerate(spec):
                if e != e_order[cur_e_pos]:
                    cur_e_pos += 1
                    issue_expert_dmas(cur_e_pos)
                kind, idx = cls[ti]
                if is8:
                    twp = _pad16(tw)
                    x_sb = x8pool.tile([128, KO, twp], f8, tag="x8_sb")
                    nc.sync.dma_start(x_sb[:], x8_d[idx][:])
                    h_sb = mm1_tile_f8(ti, e, tw, x_sb, twp)
                elif kind == "ramp":
                    x_sb = xr_sbs[idx]
                    h_sb = mm1_tile(ti, e, tw, x_sb)
                elif ti == 0:
                    x_sb = x_first
                    h_sb = mm1_tile(ti, e, tw, x_sb)
                elif kind == "nar":
                    x_sb = xnpool.tile([128, KO, TW_LAST], bf16, tag="xn_sb")
                    nc.sync.dma_start(x_sb[:], x_n[idx])
                    h_sb = mm1_tile(ti, e, tw, x_sb)
                else:
                    x_sb = xpool.tile([128, KO, CT], bf16, tag="x_sb")
                    nc.sync.dma_start(x_sb[:], x[idx])
                    h_sb = mm1_tile(ti, e, tw, x_sb)
                if (
                    ti == 0
                    and n_tiles > 1
                    and e8_order[:1]
                    and e8_order[0] == e_order[0]
                ):
                    # e0's fp8 weights are needed first of the bulk
                    # (at expert 0's tail, ~tile 5) but the gpsimd
                    # SWDGE stream delivers them marginally late on
                    # some cores. Ship them on the (otherwise idle)
                    # scalar queue instead, WAR-anchored behind tile
                    # 0's first h chunk so they stay out of the
                    # startup window.
                    e2 = e8_order[0]
                    w8_issued.add(e2)
                    nc.vector.tensor_copy(anchor[:, 20:21], h_sb[:, 0, 0:1])
                    nc.vector.tensor_add(
                        anchor[:, 21:22],
                        w8[e2][0][:, 0, 0, 0, 0:1],
                        anchor[:, 20:21],
                    )
                    nc.vector.tensor_add(
                        anchor[:, 22:23],
                        w8[e2][1][:, 0, 0, 0, 0:1],
                        anchor[:, 20:21],
                    )
                    nc.scalar.dma_start(w8[e2][0][:], w1q[e2])
                    nc.scalar.dma_start(w8[e2][1][:], w2q[e2])
                anchor_ti = 1 if n_tiles > 1 else 0
                if ti == anchor_ti:
                    # WAR anchor gating the HEAD of the gpsimd bulk
                    # stream (everything later on that queue is held
                    # behind it by FIFO order). Only e1's fp8 weights
                    # ride here; the rest of the bulk is staggered by
                    # need (tiles 2/3 + expert boundaries) so the
                    # early HBM window stays reserved for the x/w
                    # startup stream - the HBM pair budget in the
                    # first ~30 us is the binding constraint.
                    nc.vector.tensor_copy(anchor[:, 0:1], h_sb[:, 0, 0:1])
                    if len(e_order) > 1 and e_has8[e_order[1]]:
                        e2 = e_order[1]
                        nc.vector.tensor_add(
                            anchor[:, 1:2], w8[e2][0][:, 0, 0, 0, 0:1],
                            anchor[:, 0:1],
                        )
                        nc.vector.tensor_add(
                            anchor[:, 2:3], w8[e2][1][:, 0, 0, 0, 0:1],
                            anchor[:, 0:1],
                        )
                        w8_issued.add(e2)
                        nc.gpsimd.dma_start(w8[e2][0][:], w1q[e2])
                        nc.gpsimd.dma_start(w8[e2][1][:], w2q[e2])
                    elif len(e_order) > 1:
                        e2 = e_order[1]
                        we[e2] = alloc_we()
                        we_issued.add(e2)
                        nc.vector.tensor_add(
                            anchor[:, 1:2], we[e2][0][:, 0, 0, 0:1],
                            anchor[:, 0:1],
                        )
                        nc.vector.tensor_add(
                            anchor[:, 2:3], we[e2][1][:, 0, 0, 0:1],
                            anchor[:, 0:1],
                        )
                        nc.gpsimd.dma_start(we[e2][0][:], w1[e2])
                        nc.gpsimd.dma_start(we[e2][1][:], w2[e2])
                if ti == 2 and len(e_order) > 1 and n_tiles > 4:
                    e2 = e_order[1]
                    if e2 not in we_issued:
                        we_issued.add(e2)
                        if e2 not in we:
                            we[e2] = alloc_we()
                        nc.gpsimd.dma_start(we[e2][0][:], w1[e2])
                        nc.gpsimd.dma_start(we[e2][1][:], w2[e2])
                if ti == 3 and len(e_order) > 2 and n_tiles > 5:
                    e2 = e_order[2]
                    if e2 not in we_issued:
                        we_issued.add(e2)
                        if e2 not in we:
                            we[e2] = alloc_we()
                        nc.gpsimd.dma_start(we[e2][0][:], w1[e2])
                        nc.gpsimd.dma_start(we[e2][1][:], w2[e2])
                if prev is not None:
                    pti, pe, ptw, ph_sb, pis8 = prev
                    if pis8:
                        mm2_tile_f8(pti, pe, ptw, ph_sb)
                    else:
                        mm2_tile(pti, pe, ptw, ph_sb)
                prev = (ti, e, tw, h_sb, is8)
            pti, pe, ptw, ph_sb, pis8 = prev
            if pis8:
                mm2_tile_f8(pti, pe, ptw, ph_sb)
            else:
                mm2_tile(pti, pe, ptw, ph_sb)

    nc.compile()
    return nc


def kernel(x, gate_w, w1, b1, w2, b2):
    from concourse.bass_utils import run_bass_kernel_spmd

    global LAST_RESULTS

    x = np.asarray(x, dtype=np.float32)
    gate_w = np.asarray(gate_w, dtype=np.float32)
    w1 = np.asarray(w1, dtype=np.float32)
    b1 = np.asarray(b1, dtype=np.float32)
    w2 = np.asarray(w2, dtype=np.float32)
    b2 = np.asarray(b2, dtype=np.float32)

    B, S, Din = x.shape
    assert Din == D and gate_w.shape == (D, E)
    T = B * S
    xf = x.reshape(T, D)

    # ---- Host router + dispatch ----
    logits = xf.astype(np.float64) @ gate_w.astype(np.float64)
    idx0 = np.argmax(logits, axis=1)
    rows = np.arange(T)
    v0 = logits[rows, idx0]
    l2 = logits.copy()
    l2[rows, idx0] = -np.inf
    idx1 = np.argmax(l2, axis=1)
    v1_ = l2[rows, idx1]
    e1 = np.exp(v1_ - v0)
    cw0 = 1.0 / (1.0 + e1)
    cw1 = e1 / (1.0 + e1)

    token_ids = []     # bf16 pairs per expert
    combine_w = []
    token_ids8 = []    # fp8 pairs per expert
    combine_w8 = []
    for e in range(E):
        sel0 = idx0 == e
        sel1 = idx1 == e
        ids = np.nonzero(sel0 | sel1)[0]
        w = np.where(sel0[ids], cw0[ids], cw1[ids])
        m8 = w < TAU
        # tiny fp8 groups aren't worth a tile
        if m8.sum() < 64:
            m8[:] = False
        token_ids.append(ids[~m8])
        combine_w.append(w[~m8])
        token_ids8.append(ids[m8])
        combine_w8.append(w[m8])

    spec = []
    for e in range(E):
        bf_tiles = _balanced_tiles(
            len(token_ids[e]),
            n_narrow=(N_NARROW if e == E - 1 else 0),
            n_ramp=(N_RAMP if e == 0 else 0),
        )
        f8_tiles = _balanced_tiles(len(token_ids8[e]))
        if e == E - 1:
            n_nar_e = 0
            while n_nar_e < len(bf_tiles) and bf_tiles[len(bf_tiles) - 1 - n_nar_e][1] <= TW_LAST:
                n_nar_e += 1
            big_part = bf_tiles[: len(bf_tiles) - n_nar_e]
            nar_part = bf_tiles[len(bf_tiles) - n_nar_e :]
            for off, tw in big_part:
                spec.append((e, off, tw, False))
            for off, tw in f8_tiles:
                spec.append((e, off, tw, True))
            for off, tw in nar_part:
                spec.append((e, off, tw, False))
        else:
            for off, tw in bf_tiles:
                spec.append((e, off, tw, False))
            for off, tw in f8_tiles:
                spec.append((e, off, tw, True))
    spec = tuple(spec)
    n_tiles = len(spec)
    n_ramp, n_nar = _classify(spec)
    bf_specs = [(i, s) for i, s in enumerate(spec) if not s[3]]
    f8_specs = [(i, s) for i, s in enumerate(spec) if s[3]]
    n_big = len(bf_specs) - n_ramp - n_nar

    b1_zero = bool(np.all(b1 == 0.0))
    key = (spec, b1_zero)
    if key not in _NC_CACHE:
        _NC_CACHE[key] = _build(spec, b1_zero)
    nc = _NC_CACHE[key]

    # ---- Shared x tiles; per-core weight slices ----
    xr_tiles = np.zeros((max(n_ramp, 1), 128, KO, RW), dtype=BF16)
    xtiles = np.zeros((max(n_big, 1), 128, KO, CT), dtype=BF16)
    xn_tiles = np.zeros((max(n_nar, 1), 128, KO, TW_LAST), dtype=BF16)
    x8_tiles = {}
    jbf = 0
    j8 = 0
    for ti, (e, off, tw, is8) in enumerate(spec):
        if is8:
            ids_seg = token_ids8[e][off : off + tw]
            twp = _pad16(tw)
            blk = np.zeros((128, KO, twp), dtype=F8)
            xq = np.clip(xf[ids_seg], -240, 240).astype(F8)
            blk[:, :, :tw] = xq.reshape(tw, KO, 128).transpose(2, 1, 0)
            x8_tiles[f"x8_{j8}"] = np.ascontiguousarray(blk)
            j8 += 1
            continue
        ids_seg = token_ids[e][off : off + tw]
        blk = xf[ids_seg].astype(BF16).reshape(tw, KO, 128).transpose(2, 1, 0)
        if jbf < n_ramp:
            xr_tiles[jbf, :, :, :tw] = blk
        elif jbf >= len(bf_specs) - n_nar:
            xn_tiles[jbf - (len(bf_specs) - n_nar), :, :, :tw] = blk
        else:
            xtiles[jbf - n_ramp, :, :, :tw] = blk
        jbf += 1
    xr_tiles = np.ascontiguousarray(xr_tiles)
    xtiles = np.ascontiguousarray(xtiles)
    xn_tiles = np.ascontiguousarray(xn_tiles)

    b1f = b1.astype(np.float32)
    in_maps = []
    for h in range(N_CORES):
        sl = slice(h * FL, (h + 1) * FL)
        w1c = np.stack(
            [
                w1[e][:, sl]
                .reshape(KO, 128, FLO, 128)
                .transpose(1, 2, 0, 3)
                for e in range(E)
            ]
        ).astype(BF16)  # [E, 128, FLO, KO, 128]
        w2c = np.stack(
            [
                w2[e][sl, :]
                .reshape(FLO, 128, KO, 128)
                .transpose(1, 2, 0, 3)
                for e in range(E)
            ]
        ).astype(BF16)  # [E, 128, KO, FLO, 128]
        # fp8 copies, pre-scaled by WS, DoubleRow-pair layouts
        w1qc = np.stack(
            [
                np.clip(w1[e][:, sl] * WS, -240, 240)
                .astype(F8)
                .reshape(KO // 2, 2, 128, FLO, 128)
                .transpose(2, 3, 0, 1, 4)
                for e in range(E)
            ]
        )  # [E, 128, FLO, KO//2, 2, 128]
        w2qc = np.stack(
            [
                np.clip(w2[e][sl, :] * WS, -240, 240)
                .astype(F8)
                .reshape(FLO // 2, 2, 128, KO, 128)
                .transpose(2, 3, 0, 1, 4)
                for e in range(E)
            ]
        )  # [E, 128, KO, FLO//2, 2, 128]
        b1c = np.stack(
            [b1f[e][sl].reshape(FLO, 128).T for e in range(E)], axis=1
        ).reshape(128, E * FLO)  # [128, E*FLO]
        m = {
            "x_r": xr_tiles,
            "x": xtiles,
            "x_n": xn_tiles,
            "w1": np.ascontiguousarray(w1c),
            "w2": np.ascontiguousarray(w2c),
            "w1q": np.ascontiguousarray(w1qc),
            "w2q": np.ascontiguousarray(w2qc),
            "b1": np.ascontiguousarray(b1c),
        }
        m.update(x8_tiles)
        in_maps.append(m)

    res = run_bass_kernel_spmd(nc, in_maps, core_ids=list(range(N_CORES)))
    LAST_RESULTS = res

    # ---- Host: sum the 8 F-slice partials, combine, scatter ----
    def summed(name):
        s = res.results[0][name].astype(np.float32)
        for h in range(1, N_CORES):
            s = s + res.results[h][name].astype(np.float32)
        return s

    yr_sum = summed("y_r")
    ysum = summed("y")
    y2sum = summed("y2")
    y8sum = {j: summed(f"y8_{j}") for j in range(len(f8_specs))}

    out = np.zeros((T, D), dtype=np.float32)
    jbf = 0
    j8 = 0
    for ti, (e, off, tw, is8) in enumerate(spec):
        if is8:
            ids_seg = token_ids8[e][off : off + tw]
            cw_seg = combine_w8[e][off : off + tw].astype(np.float32)
            yt = y8sum[j8][:, :, :tw].transpose(2, 1, 0).reshape(tw, D)
            out[ids_seg] += cw_seg[:, None] * (yt * np.float32(1.0 / WS) + b2[e])
            j8 += 1
            continue
        ids_seg = token_ids[e][off : off + tw]
        cw_seg = combine_w[e][off : off + tw].astype(np.float32)
        if jbf < n_ramp:
            yt = yr_sum[jbf, :, :, :tw]
        elif jbf >= len(bf_specs) - n_nar:
            yt = y2sum[jbf - (len(bf_specs) - n_nar), :, :, :tw]
        else:
            yt = ysum[jbf - n_ramp, :, :, :tw]
        yt = yt.transpose(2, 1, 0).reshape(tw, D)
        out[ids_seg] += cw_seg[:, None] * (yt + b2[e])
        jbf += 1

    return out.reshape(B, S, D)
